# revision 85
# baseline (speedup 1.0000x reference)
"""Trainium2 Bass kernel for nn_ASD_RNN (encoder + fused-gate LSTM + prototype-distance head).

Contract: kernel(**inputs) takes FULL unsharded inputs (as in reference.setup_inputs())
and returns the FULL [64, 1] float32 output. Internally shards batch across 8 cores
(8 batches per core), runs one Bass kernel per core via run_bass_kernel_spmd, gathers.

Per-core layout (BC = 8 batches, R = BC*S = 512 rows, s-major: r = s*8 + b):
  - All GEMM operands are bf16; PSUM accumulation is fp32.
  - v is shipped pre-transposed from the host: vT [f%128, f//128, r] feeds the
    encoder and prototype-distance GEMMs; a row-major copy vrow feeds ||v||^2.
  - Encoder: fuseT[h%128, h//128, r] = relu(Wenc^T v + benc) + cat_emb one-hot fold.
  - xg = fuse @ Wx + (bx+bh), computed row-major into SBUF-resident xgsb
    [r%128, r//128, g] (bias added via a host-replicated [128, G] tensor).
  - LSTM keeps h transposed (hT [h%128, k, b]) as the matmul stationary; Wh is the
    moving operand (one full Wh pass per step is the PE floor). The per-step xg+bias
    contribution is folded into the gates PSUM with a 32-row selector matmul
    (eyevar) so no per-step DMA or staging copies are needed.
  - Distance head: ||v-p||^2 via matmul folds (-2p | W_gate rides as column 20),
    log-ratio via two Ln activations with per-partition bias, reduced via a
    selector matmul.
"""

import numpy as np
import ml_dtypes

import concourse.bass as bass
import concourse.mybir as mybir
import concourse.tile as tile
from concourse import bacc
from concourse.bass_utils import run_bass_kernel_spmd

AF = mybir.ActivationFunctionType
ALU = mybir.AluOpType
DT = mybir.dt
AX = mybir.AxisListType

B, S, F, H, P2 = 64, 64, 2048, 512, 20
G = 4 * H
NCORES = 8
BC = B // NCORES          # 8 batches per core
R = BC * S                # 512 rows per core
KF = F // 128             # 16 k-tiles over F
KH = H // 128             # 4 k-tiles over H
MR = R // 128             # 4 row tiles
F32 = DT.float32
BF = DT.bfloat16
FP8 = DT.float8e4
MMF = DT.float32r
BF_NP = ml_dtypes.bfloat16
FP8_NP = ml_dtypes.float8_e4m3
WH_SCALE = 32.0  # Wh/W_dec stored fp8 as x32; pre-acts carry x32, acts undo

import os as _os
# "mix": k-tiles 0-1 via one fp8-DoubleRow matmul, k-tiles 2-3 bf16 — cuts
# the Wh stream 25% at half the PE cell-activity of pure DR (which trips the
# power limiter). "bf16" = fully unthrottled fallback.
KWH = _os.environ.get("KWH", "mix")
KDUM = int(_os.environ.get("KDUM", "0"))
KOSPLIT = int(_os.environ.get("KOSPLIT", "0"))
KFILL = int(_os.environ.get("KFILL", "0"))


def build_nc():
    nc = bacc.Bacc("TRN2", target_bir_lowering=False, debug=False,
                   num_devices=NCORES)

    def din(name, shape, dt=BF):
        return nc.dram_tensor(name, shape, dt, kind="ExternalInput").ap()

    vT_d = din("vT", [128, KF, R])
    vrow_d = din("vrow", [128, MR, F])
    wenc_d = din("wencp", [128, KF, H])
    benc_d = din("benc", [128, KH], F32)
    catemb_d = din("catemb", [3, H])
    onehot_d = din("onehot", [3, R])
    wx_d = din("wxp", [128, KH, G])
    bxh_d = din("bxh128", [128, G], F32)
    wh_d = din("whp8", [128, KH, G], BF)
    wh8_d = din("wh8", [128, 2, G], FP8)
    wdec8_d = din("wdec8", [128, 2], FP8)
    eyevar_d = din("eyevar", [128, 8, 32])
    eye8_d = din("eye8", [BC, BC])
    pwT_d = din("pwT", [128, KF, 22])
    pprow_d = din("pprow", [1, 22], MMF)
    ones1_d = din("ones1", [1, 128], MMF)
    wddr_d = din("wddr", [128, MR, P2], F32)
    sel8_d = din("sel8", [128, BC], MMF)
    wdec_d = din("wdecp8", [128, KH], BF)
    b3bc_d = din("b3bc", [BC, 3], F32)
    out_d = nc.dram_tensor("out", [BC, 1], F32, kind="ExternalOutput").ap()

    with tile.TileContext(nc) as tc:
        _body(tc, nc, vT_d, vrow_d, wenc_d, benc_d, catemb_d, onehot_d, wx_d,
              bxh_d, wh_d, wh8_d, wdec8_d, eyevar_d, eye8_d, pwT_d, pprow_d,
              ones1_d, wddr_d, sel8_d, wdec_d, b3bc_d, out_d)
    nc.compile()
    return nc


def _body(tc, nc, vT_d, vrow_d, wenc_d, benc_d, catemb_d, onehot_d, wx_d,
          bxh_d, wh_d, wh8_d, wdec8_d, eyevar_d, eye8_d, pwT_d, pprow_d,
          ones1_d, wddr_d, sel8_d, wdec_d, b3bc_d, out_d):
    import os
    PHASES = int(os.environ.get("KPHASES", "9"))
    with tc.tile_pool(name="persist", bufs=1) as P:
        vT = P.tile([128, KF, R], BF)
        wencp = P.tile([128, KF, H], BF)
        wxp = P.tile([128, KH, G], BF)
        whsb = P.tile([128, KH, G], BF)
        wh8sb = P.tile([128, 2, G], FP8)
        wdec8p = P.tile([128, 2], FP8)
        fuseT = P.tile([128, KH, R], BF)
        xgsb = P.tile([128, MR, G], BF)
        bxh128 = P.tile([128, G], F32)
        catemb = P.tile([3, H], BF)
        onehot = P.tile([3, R], BF)
        benc = P.tile([128, KH], F32)
        pwT = P.tile([128, KF, 22], BF)
        pprow = P.tile([1, 22], MMF)
        ones1f = P.tile([1, 128], MMF)
        wddr = P.tile([128, MR, P2], F32)
        sel8 = P.tile([128, BC], MMF)
        eyevar = P.tile([128, 8, 32], BF)
        eye8 = P.tile([BC, BC], BF)
        wdecp = P.tile([128, KH], BF)
        b3bc = P.tile([BC, 3], F32)
        hT = P.tile([128, KH, BC], BF)
        # 32-wide out-partition padding: DoubleRow needs >=32 stationary cols
        hT8 = P.tile([128, 2, 32], FP8)
        cst = P.tile([BC, H], BF)
        vv = P.tile([128, MR], F32)
        vve = P.tile([128, MR], F32)
        grs = P.tile([128, MR, 2], MMF)
        osb = P.tile([BC, 1], F32)
        gsb = P.tile([BC, 1], F32)
        dsb = P.tile([BC, 1], F32)
        fin = P.tile([BC, 1], F32)

        # ---- input DMAs (queue order = priority: encoder set first; vT and
        # wencp chunked by ko so the encoder k-loop can start early) ----
        for c in range(4):
            nc.sync.dma_start(vT[:, 4 * c:4 * (c + 1), :],
                              vT_d[:, 4 * c:4 * (c + 1), :])
        for c in range(2):
            nc.sync.dma_start(wencp[:, 8 * c:8 * (c + 1), :],
                              wenc_d[:, 8 * c:8 * (c + 1), :])
        nc.sync.dma_start(benc, benc_d)
        nc.sync.dma_start(catemb, catemb_d)
        nc.sync.dma_start(onehot, onehot_d)
        nc.sync.dma_start(wxp, wx_d)
        nc.sync.dma_start(bxh128, bxh_d)
        nc.sync.dma_start(whsb, wh_d)
        nc.sync.dma_start(wh8sb, wh8_d)
        nc.sync.dma_start(wdec8p, wdec8_d)
        nc.sync.dma_start(eyevar, eyevar_d)
        nc.sync.dma_start(eye8, eye8_d)
        nc.sync.dma_start(pwT, pwT_d)
        nc.sync.dma_start(pprow, pprow_d)
        nc.sync.dma_start(ones1f, ones1_d)
        nc.sync.dma_start(wddr, wddr_d)
        nc.sync.dma_start(sel8, sel8_d)
        nc.sync.dma_start(wdecp, wdec_d)
        nc.sync.dma_start(b3bc, b3bc_d)

        # ---- encoder: fuseT = relu(Wenc^T v + benc) + catemb-fold ----
        if PHASES >= 2:
            with tc.tile_pool(name="psf", bufs=2, space="PSUM") as psf, \
                 tc.tile_pool(name="psc", bufs=2, space="PSUM") as psc, \
                 tc.tile_pool(name="encs", bufs=2) as encs:
                for m in range(KH):
                    ps = psf.tile([128, R], F32)
                    for ko in range(KF):
                        nc.tensor.matmul(
                            ps, wencp[:, ko, m * 128:(m + 1) * 128],
                            vT[:, ko, :], start=(ko == 0), stop=(ko == KF - 1))
                    pc = psc.tile([128, R], F32)
                    nc.tensor.matmul(pc, catemb[:, m * 128:(m + 1) * 128],
                                     onehot, start=True, stop=True)
                    sc = encs.tile([128, R], F32)
                    nc.scalar.activation(sc, ps, AF.Relu,
                                         bias=benc[:, m:m + 1])
                    nc.vector.tensor_add(fuseT[:, m, :], sc, pc)

        # ---- vv[r] = sum_f v[r,f]^2 (Act engine; emitted after the encoder
        # relus so a late vrow DMA can't stall them on the in-order engine) --
        if PHASES >= 1:
            with tc.tile_pool(name="vvp", bufs=1) as VP, \
                 tc.tile_pool(name="vsq", bufs=2) as SQ:
                vrow = VP.tile([128, MR, F], BF)
                nc.sync.dma_start(vrow, vrow_d)
                for m in range(MR):
                    sq = SQ.tile([128, F], BF)
                    nc.scalar.activation(sq, vrow[:, m, :], AF.Square,
                                         accum_out=vv[:, m:m + 1])
                nc.vector.tensor_scalar_add(vve, vv, 1e-8)

        # ---- xg row-block m: 4 psum groups + x32 bias fold (shared) ----
        def xg_group(psx, m, nb):
            ps = psx.tile([128, 512], F32, tag="x")
            for k in range(KH):
                nc.tensor.matmul(
                    ps, fuseT[:, k, m * 128:(m + 1) * 128],
                    wxp[:, k, nb * 512:(nb + 1) * 512],
                    start=(k == 0), stop=(k == KH - 1))
            # xg carries x32 so it folds into the x32 fp8 Wh partials;
            # bxh128 is host-prescaled by 32.
            def fin():
                nc.vector.scalar_tensor_tensor(
                    xgsb[:, m, nb * 512:(nb + 1) * 512], ps, WH_SCALE,
                    bxh128[:, nb * 512:(nb + 1) * 512], ALU.mult, ALU.add)
            return fin

        # ---- xg ahead of the LSTM (m>0 interleaves into steps if KFILL) ----
        if PHASES >= 3:
            with tc.tile_pool(name="psx0", bufs=2, space="PSUM") as psx0:
                for mm_ in range(1 if KFILL else MR):
                    for nb in range(4):
                        xg_group(psx0, mm_, nb)()

        # ---- LSTM + interleaved distance head + xg m=1..3 + decoder ----
        if PHASES >= 5:
            NBORD = (0, 1, 3, 2)  # i, f, g, o last: c-chain unblocks earlier
            nc.vector.memset(hT.bitcast(DT.uint8), 0)
            nc.vector.memset(hT8.bitcast(DT.uint8), 0)
            with tc.tile_pool(name="psl", bufs=3 if KFILL else 4,
                              space="PSUM") as psl, \
                 tc.tile_pool(name="pstr", bufs=2, space="PSUM") as pstr, \
                 tc.tile_pool(name="psx", bufs=1, space="PSUM") as psx, \
                 tc.tile_pool(name="psq", bufs=1, space="PSUM") as psq, \
                 tc.tile_pool(name="psd", bufs=1, space="PSUM") as psd, \
                 tc.tile_pool(name="gap", bufs=6) as gap, \
                 tc.tile_pool(name="gaop", bufs=4) as gaop, \
                 tc.tile_pool(name="hp", bufs=2) as hp, \
                 tc.tile_pool(name="dfp", bufs=6) as dfp, \
                 tc.tile_pool(name="ltp", bufs=2) as ltp:
                dist_pq = [None] * MR

                def dist_half(m, half):
                    # 8 of the 16 -2*v.p matmuls for row-block m (PE filler)
                    if half == 0:
                        dist_pq[m] = psq.tile([128, 22], F32, tag="q",
                                              name=f"pq{m}")
                    pq = dist_pq[m]
                    for ko in range(8 * half, 8 * half + 8):
                        nc.tensor.matmul(pq, vT[:, ko, m * 128:(m + 1) * 128],
                                         pwT[:, ko, :],
                                         start=(ko == 0), stop=False)
                    if half == 1:
                        nc.tensor.matmul(pq, ones1f, pprow,
                                         start=False, stop=True)

                def dist_finish(m):
                    # dist_feat = log((d+1)/(d+eps)) ~= x - x^2/2, x = 1/d
                    # (d ~ 2700 so the truncation error is ~1e-11); keeps the
                    # act engine on the sigmoid table (no Ln table swap).
                    pq = dist_pq[m]
                    dd = dfp.tile([128, P2], F32)
                    nc.scalar.activation(dd, pq[:, 0:P2], AF.Identity,
                                         bias=vve[:, m:m + 1])
                    nc.scalar.copy(grs[:, m, 0:1], pq[:, P2:P2 + 1])
                    x_ = dfp.tile([128, P2], F32)
                    nc.vector.reciprocal(x_, dd)
                    t_ = dfp.tile([128, P2], F32)
                    nc.vector.tensor_scalar_mul(t_, x_, -0.5)
                    nc.vector.tensor_scalar_add(t_, t_, 1.0)
                    nc.vector.tensor_mul(t_, t_, x_)
                    nc.vector.tensor_mul(t_, t_, wddr[:, m, :])
                    with nc.allow_low_precision(reason="20-wide reduce, f32r"):
                        nc.vector.reduce_sum(out=grs[:, m, 1:2], in_=t_,
                                             axis=AX.X)

                def dist_tail():
                    pr = psd.tile([BC, 2], F32, tag="d")
                    for dm in range(MR):
                        nc.tensor.matmul(pr, sel8, grs[:, dm, :],
                                         start=(dm == 0), stop=(dm == MR - 1))
                    nc.scalar.activation(gsb, pr[:, 0:1], AF.Sigmoid,
                                         bias=b3bc[:, 1:2], scale=1.0 / S)
                    nc.scalar.activation(dsb, pr[:, 1:2], AF.Sigmoid,
                                         bias=b3bc[:, 2:3])

                if not KFILL:
                    for dm in range(MR):
                        dist_half(dm, 0)
                        dist_half(dm, 1)
                        dist_finish(dm)
                    dist_tail()

                OW = 32 if KWH == "mix" else BC

                def emit_folds(s):
                    # xg+bias fold matmuls for step s: independent of h, so
                    # they are emitted one step ahead (into the PE stream
                    # ahead of step s-1's transposes) to fill the PE stall
                    # while act(o)/h of the previous step complete.
                    m = s // 16
                    p0 = (s * 8) % 128
                    blk = (p0 // 64) * 64
                    q = (p0 % 64) // 8
                    pss = {}
                    for nb in NBORD:
                        ps = psl.tile([OW, 512], F32, tag="l", name=f"l{s%4}")
                        nhalf = 2 if (nb == 2 and KOSPLIT) else 1
                        w = 512 // nhalf
                        for hh in range(nhalf):
                            nc.tensor.matmul(
                                ps[:, hh * w:(hh + 1) * w],
                                eyevar[blk:blk + 64, q, 0:OW],
                                xgsb[blk:blk + 64, m,
                                     nb * 512 + hh * w:nb * 512 + (hh + 1) * w],
                                start=True, stop=(s == 0))
                        pss[nb] = ps
                    return pss

                pss_next = emit_folds(0)
                for s in range(S):
                    pss = pss_next
                    ga = {}
                    for nb in NBORD:
                        ps = pss[nb]
                        nhalf = 2 if (nb == 2 and KOSPLIT) else 1
                        w = 512 // nhalf
                        for hh in range(nhalf):
                            if s > 0 and KWH == "mix":
                                # k-tiles 0,1 in one fp8 DoubleRow pass
                                nc.tensor.matmul(
                                    ps[:, hh * w:(hh + 1) * w],
                                    hT8,
                                    wh8sb[:, :, nb * 512 + hh * w:
                                          nb * 512 + (hh + 1) * w],
                                    start=False, stop=False,
                                    perf_mode=mybir.MatmulPerfMode.DoubleRow)
                                for k in (2, 3):
                                    nc.tensor.matmul(
                                        ps[0:BC, hh * w:(hh + 1) * w],
                                        hT[:, k, 0:BC],
                                        whsb[:, k, nb * 512 + hh * w:
                                             nb * 512 + (hh + 1) * w],
                                        start=False, stop=(k == KH - 1),
                                        skip_group_check=True)
                            elif s > 0:
                                for k in range(KH):
                                    nc.tensor.matmul(
                                        ps[0:BC, hh * w:(hh + 1) * w],
                                        hT[:, k, 0:BC],
                                        whsb[:, k, nb * 512 + hh * w:
                                             nb * 512 + (hh + 1) * w],
                                        start=False, stop=(k == KH - 1))
                            gpool = gap if nhalf == 1 else gaop
                            g = gpool.tile([BC, w], BF)
                            nc.scalar.activation(
                                g, ps[0:BC, hh * w:(hh + 1) * w],
                                AF.Tanh if nb == 3 else AF.Sigmoid,
                                scale=1.0 / WH_SCALE)
                            ga.setdefault(nb, []).append(g)
                    if s == 0:
                        nc.vector.tensor_mul(cst, ga[0][0], ga[3][0])
                    else:
                        t1 = ltp.tile([BC, H], BF)
                        nc.vector.tensor_mul(t1, ga[0][0], ga[3][0])  # i*g
                        nc.vector.tensor_mul(cst, cst, ga[1][0])      # f*c
                        nc.vector.tensor_add(cst, cst, t1)
                    if s + 1 < S:
                        pss_next = emit_folds(s + 1)
                    # PE filler between the step's matmuls and transposes:
                    # real work where available, else p-state keepalive.
                    fins = []
                    if KFILL:
                        if s < 12:
                            fins.append(xg_group(psx, 1 + s // 4, s % 4))
                        elif s < 20:
                            dm, dh = (s - 12) // 2, (s - 12) % 2
                            dist_half(dm, dh)
                            if dh == 1:
                                fins.append(lambda dm=dm: dist_finish(dm))
                        else:
                            for _ in range(KDUM):
                                dps = psx.tile([128, 512], F32, tag="x")
                                nc.tensor.matmul(dps, fuseT[:, 0, 0:128],
                                                 wxp[:, 0, 0:512],
                                                 start=True, stop=True)
                    # h = o*c in k-chunks so transpose/cast pipeline per k;
                    # casts alternate DVE/Act to halve the serial tail.
                    h = hp.tile([BC, H], BF)
                    for k in range(KH):
                        if len(ga[2]) == 2:
                            osrc = ga[2][k // 2][:, (k % 2) * 128:
                                                 (k % 2 + 1) * 128]
                        else:
                            osrc = ga[2][0][:, k * 128:(k + 1) * 128]
                        nc.vector.tensor_mul(
                            h[:, k * 128:(k + 1) * 128], osrc,
                            cst[:, k * 128:(k + 1) * 128])
                        pt = pstr.tile([128, BC], BF, tag="tr")
                        nc.tensor.transpose(pt, h[:, k * 128:(k + 1) * 128],
                                            eye8)
                        # all casts on DVE: a cast on the in-order act engine
                        # would block the next step's gate activations
                        if KWH == "mix" and k < 2:
                            nc.vector.tensor_copy(hT8[:, k, 0:BC], pt)
                        else:
                            nc.vector.tensor_copy(hT[:, k, 0:BC], pt)
                    for fcb in fins:
                        fcb()
                    if KFILL and s == 21:
                        dist_tail()
                # decoder
                pd = psd.tile([BC, 2], F32, tag="d")
                for k in range(KH):
                    if KWH == "mix" and k < 2:
                        nc.tensor.matmul(pd[:, 0:1], hT8[:, k, 0:BC],
                                         wdec8p[:, k:k + 1],
                                         start=(k == 0), stop=False)
                    else:
                        nc.tensor.matmul(pd[:, 0:1], hT[:, k, 0:BC],
                                         wdecp[:, k:k + 1],
                                         start=(k == 0), stop=(k == KH - 1))
                nc.scalar.activation(osb, pd[:, 0:1], AF.Sigmoid,
                                     bias=b3bc[:, 0:1], scale=1.0 / WH_SCALE)

        # ---- combine ----
        if PHASES >= 6:
            nc.vector.tensor_sub(fin, osb, dsb)
            nc.vector.scalar_tensor_tensor(fin, fin, gsb[:, 0:1], dsb,
                                           ALU.mult, ALU.add)
            nc.sync.dma_start(out_d, fin)


_NC_CACHE = {}


def _get_nc():
    if "nc" not in _NC_CACHE:
        _NC_CACHE["nc"] = build_nc()
    return _NC_CACHE["nc"]


def _make_in_maps(v_feat, category, W_enc, b_enc, Wx, bx, Wh, bh, cat_emb,
                  W_dec, b_dec, prototype, W_dd, b_dd, W_gate, b_gate):
    f32 = np.float32
    v_feat = np.asarray(v_feat, f32)
    category = np.asarray(category).astype(np.int64)

    wencp = np.ascontiguousarray(
        np.asarray(W_enc, f32).reshape(KF, 128, H).transpose(1, 0, 2)
    ).astype(BF_NP)
    benc = np.ascontiguousarray(
        np.asarray(b_enc, f32).reshape(KH, 128).T).copy()
    catemb = np.asarray(cat_emb, f32).astype(BF_NP)
    wxp = np.ascontiguousarray(
        np.asarray(Wx, f32).reshape(KH, 128, G).transpose(1, 0, 2)
    ).astype(BF_NP)
    bxh128 = np.ascontiguousarray(
        np.tile(WH_SCALE * (np.asarray(bx, f32)
                            + np.asarray(bh, f32)).reshape(1, G),
                (128, 1)))
    whs = (WH_SCALE * np.asarray(Wh, f32)).reshape(KH, 128, G)
    whp8 = np.ascontiguousarray(whs.transpose(1, 0, 2)).astype(BF_NP)
    wh8 = np.ascontiguousarray(whs[0:2].transpose(1, 0, 2)).astype(FP8_NP)
    # eyevar[p, q, j] = 1 iff p%64 == q*8+j (64-aligned step-row selector);
    # cols 8..31 are zero padding so the 32-wide PSUM region is fully started.
    pp_ = np.arange(128)
    eyevar = np.zeros((128, 8, 32), f32)
    for qq in range(8):
        for j in range(BC):
            eyevar[pp_ % 64 == qq * 8 + j, qq, j] = 1.0
    eyevar = eyevar.astype(BF_NP)
    eye8 = np.eye(BC, dtype=f32).astype(BF_NP)
    proto = np.asarray(prototype, f32)
    pw = np.concatenate([-2.0 * proto,
                         np.asarray(W_gate, f32).reshape(1, F),
                         np.zeros((1, F), f32)], axis=0)  # [22, F]
    pwT = np.ascontiguousarray(
        pw.T.reshape(KF, 128, 22).transpose(1, 0, 2)).astype(BF_NP)
    pprow = np.concatenate([(proto * proto).sum(axis=1),
                            np.zeros(2, f32)]).reshape(1, 22).astype(f32)
    ones1 = np.ones((1, 128), f32)
    wdd = np.asarray(W_dd, f32).reshape(S, P2)
    rep = np.repeat(wdd[:, None, :], BC, axis=1).reshape(R, P2)
    wddr = np.ascontiguousarray(rep.reshape(MR, 128, P2).transpose(1, 0, 2))
    sel8 = np.zeros((128, BC), f32)
    sel8[np.arange(128), np.arange(128) % BC] = 1.0
    wdecs = (WH_SCALE * np.asarray(W_dec, f32)).reshape(KH, 128).T
    wdecp8 = np.ascontiguousarray(wdecs).astype(BF_NP)
    wdec8 = np.ascontiguousarray(wdecs[:, 0:2]).astype(FP8_NP)
    b3 = np.array([np.asarray(b_dec, f32).reshape(-1)[0],
                   np.asarray(b_gate, f32).reshape(-1)[0],
                   np.asarray(b_dd, f32).reshape(-1)[0]], f32)
    b3bc = np.ascontiguousarray(np.tile(b3.reshape(1, 3), (BC, 1)))

    common = {
        "wencp": wencp, "benc": benc, "catemb": catemb, "wxp": wxp,
        "bxh128": bxh128, "whp8": whp8, "wh8": wh8, "wdec8": wdec8,
        "eyevar": eyevar, "eye8": eye8,
        "pwT": pwT, "pprow": pprow, "ones1": ones1, "wddr": wddr,
        "sel8": sel8, "wdecp8": wdecp8, "b3bc": b3bc,
    }
    in_maps = []
    for j in range(NCORES):
        vs = np.ascontiguousarray(
            v_feat[j * BC:(j + 1) * BC].transpose(1, 0, 2).reshape(R, F))
        vTn = np.ascontiguousarray(
            vs.reshape(R, KF, 128).transpose(2, 1, 0)).astype(BF_NP)
        vrow = np.ascontiguousarray(
            vs.reshape(MR, 128, F).transpose(1, 0, 2)).astype(BF_NP)
        cats = category[j * BC:(j + 1) * BC]
        onehot = (cats[None, :] == np.arange(3)[:, None]).astype(f32)
        onehot = np.ascontiguousarray(
            np.tile(onehot, (1, S))).astype(BF_NP)  # [3, R], r = s*8+b
        in_maps.append({"vT": vTn, "vrow": vrow, "onehot": onehot, **common})
    return in_maps


def run(trace=False, **inputs):
    nc = _get_nc()
    in_maps = _make_in_maps(**inputs)
    res = run_bass_kernel_spmd(nc, in_maps, list(range(NCORES)), trace=trace)
    out = np.concatenate([res.results[j]["out"] for j in range(NCORES)],
                         axis=0).astype(np.float32)
    return out, res


def kernel(**inputs):
    out, _ = run(trace=False, **inputs)
    return out


# revision 87
# speedup vs baseline: 1.0290x; 1.0290x over previous
"""Trainium2 Bass kernel for nn_ASD_RNN (encoder + fused-gate LSTM + prototype-distance head).

Contract: kernel(**inputs) takes FULL unsharded inputs (as in reference.setup_inputs())
and returns the FULL [64, 1] float32 output. Internally shards batch across 8 cores
(8 batches per core), runs one Bass kernel per core via run_bass_kernel_spmd, gathers.

Per-core layout (BC = 8 batches, R = BC*S = 512 rows, s-major: r = s*8 + b):
  - All GEMM operands are bf16; PSUM accumulation is fp32.
  - v is shipped pre-transposed from the host: vT [f%128, f//128, r] feeds the
    encoder and prototype-distance GEMMs; a row-major copy vrow feeds ||v||^2.
  - Encoder: fuseT[h%128, h//128, r] = relu(Wenc^T v + benc) + cat_emb one-hot fold.
  - xg = fuse @ Wx + (bx+bh), computed row-major into SBUF-resident xgsb
    [r%128, r//128, g] (bias added via a host-replicated [128, G] tensor).
  - LSTM keeps h transposed (hT [h%128, k, b]) as the matmul stationary; Wh is the
    moving operand (one full Wh pass per step is the PE floor). The per-step xg+bias
    contribution is folded into the gates PSUM with a 32-row selector matmul
    (eyevar) so no per-step DMA or staging copies are needed.
  - Distance head: ||v-p||^2 via matmul folds (-2p | W_gate rides as column 20),
    log-ratio via two Ln activations with per-partition bias, reduced via a
    selector matmul.
"""

import numpy as np
import ml_dtypes

import concourse.bass as bass
import concourse.mybir as mybir
import concourse.tile as tile
from concourse import bacc
from concourse.bass_utils import run_bass_kernel_spmd

AF = mybir.ActivationFunctionType
ALU = mybir.AluOpType
DT = mybir.dt
AX = mybir.AxisListType

B, S, F, H, P2 = 64, 64, 2048, 512, 20
G = 4 * H
NCORES = 8
BC = B // NCORES          # 8 batches per core
R = BC * S                # 512 rows per core
KF = F // 128             # 16 k-tiles over F
KH = H // 128             # 4 k-tiles over H
MR = R // 128             # 4 row tiles
F32 = DT.float32
BF = DT.bfloat16
FP8 = DT.float8e4
MMF = DT.float32r
BF_NP = ml_dtypes.bfloat16
FP8_NP = ml_dtypes.float8_e4m3
WH_SCALE = 32.0  # Wh/W_dec stored fp8 as x32; pre-acts carry x32, acts undo

import os as _os
# "bf16" (default): fully unthrottled, fastest measured. "mix" (k-tiles 0-1
# via fp8-DoubleRow) cuts the Wh stream 25% but still trips the PE power
# limiter and measures slower.
KWH = _os.environ.get("KWH", "bf16")
KDUM = int(_os.environ.get("KDUM", "0"))
KOSPLIT = int(_os.environ.get("KOSPLIT", "0"))
KFILL = int(_os.environ.get("KFILL", "0"))


def build_nc():
    nc = bacc.Bacc("TRN2", target_bir_lowering=False, debug=False,
                   num_devices=NCORES)

    def din(name, shape, dt=BF):
        return nc.dram_tensor(name, shape, dt, kind="ExternalInput").ap()

    vT_d = din("vT", [128, KF, R])
    vrow_d = din("vrow", [128, MR, F])
    wenc_d = din("wencp", [128, KF, H])
    benc_d = din("benc", [128, KH], F32)
    catemb_d = din("catemb", [3, H])
    onehot_d = din("onehot", [3, R])
    wx_d = din("wxp", [128, KH, G])
    bxh_d = din("bxh128", [128, G], F32)
    wh_d = din("whp8", [128, KH, G], BF)
    wh8_d = din("wh8", [128, 2, G], FP8)
    wdec8_d = din("wdec8", [128, 2], FP8)
    eyevar_d = din("eyevar", [128, 8, 32])
    eye8_d = din("eye8", [BC, BC])
    pwT_d = din("pwT", [128, KF, 22])
    pprow_d = din("pprow", [1, 22], MMF)
    ones1_d = din("ones1", [1, 128], MMF)
    wddr_d = din("wddr", [128, MR, P2], F32)
    sel8_d = din("sel8", [128, BC], MMF)
    wdec_d = din("wdecp8", [128, KH], BF)
    b3bc_d = din("b3bc", [BC, 3], F32)
    out_d = nc.dram_tensor("out", [BC, 1], F32, kind="ExternalOutput").ap()

    with tile.TileContext(nc) as tc:
        _body(tc, nc, vT_d, vrow_d, wenc_d, benc_d, catemb_d, onehot_d, wx_d,
              bxh_d, wh_d, wh8_d, wdec8_d, eyevar_d, eye8_d, pwT_d, pprow_d,
              ones1_d, wddr_d, sel8_d, wdec_d, b3bc_d, out_d)
    nc.compile()
    return nc


def _body(tc, nc, vT_d, vrow_d, wenc_d, benc_d, catemb_d, onehot_d, wx_d,
          bxh_d, wh_d, wh8_d, wdec8_d, eyevar_d, eye8_d, pwT_d, pprow_d,
          ones1_d, wddr_d, sel8_d, wdec_d, b3bc_d, out_d):
    import os
    PHASES = int(os.environ.get("KPHASES", "9"))
    with tc.tile_pool(name="persist", bufs=1) as P:
        vT = P.tile([128, KF, R], BF)
        wencp = P.tile([128, KF, H], BF)
        wxp = P.tile([128, KH, G], BF)
        whsb = P.tile([128, KH, G], BF)
        wh8sb = P.tile([128, 2, G], FP8)
        wdec8p = P.tile([128, 2], FP8)
        fuseT = P.tile([128, KH, R], BF)
        xgsb = P.tile([128, MR, G], BF)
        bxh128 = P.tile([128, G], F32)
        catemb = P.tile([3, H], BF)
        onehot = P.tile([3, R], BF)
        benc = P.tile([128, KH], F32)
        pwT = P.tile([128, KF, 22], BF)
        pprow = P.tile([1, 22], MMF)
        ones1f = P.tile([1, 128], MMF)
        wddr = P.tile([128, MR, P2], F32)
        sel8 = P.tile([128, BC], MMF)
        eyevar = P.tile([128, 8, 32], BF)
        eye8 = P.tile([BC, BC], BF)
        wdecp = P.tile([128, KH], BF)
        b3bc = P.tile([BC, 3], F32)
        hT = P.tile([128, KH, BC], BF)
        # 32-wide out-partition padding: DoubleRow needs >=32 stationary cols
        hT8 = P.tile([128, 2, 32], FP8)
        cst = P.tile([BC, H], BF)
        vv = P.tile([128, MR], F32)
        vve = P.tile([128, MR], F32)
        grs = P.tile([128, MR, 2], MMF)
        osb = P.tile([BC, 1], F32)
        gsb = P.tile([BC, 1], F32)
        dsb = P.tile([BC, 1], F32)
        fin = P.tile([BC, 1], F32)

        # ---- input DMAs (queue order = priority: encoder set first; vT and
        # wencp chunked by ko so the encoder k-loop can start early) ----
        for c in range(4):
            nc.sync.dma_start(vT[:, 4 * c:4 * (c + 1), :],
                              vT_d[:, 4 * c:4 * (c + 1), :])
        for c in range(2):
            nc.sync.dma_start(wencp[:, 8 * c:8 * (c + 1), :],
                              wenc_d[:, 8 * c:8 * (c + 1), :])
        nc.sync.dma_start(benc, benc_d)
        nc.sync.dma_start(catemb, catemb_d)
        nc.sync.dma_start(onehot, onehot_d)
        nc.sync.dma_start(wxp, wx_d)
        nc.sync.dma_start(bxh128, bxh_d)
        nc.sync.dma_start(whsb, wh_d)
        if KWH == "mix":
            nc.sync.dma_start(wh8sb, wh8_d)
            nc.sync.dma_start(wdec8p, wdec8_d)
        nc.sync.dma_start(eyevar, eyevar_d)
        nc.sync.dma_start(eye8, eye8_d)
        nc.sync.dma_start(pwT, pwT_d)
        nc.sync.dma_start(pprow, pprow_d)
        nc.sync.dma_start(ones1f, ones1_d)
        nc.sync.dma_start(wddr, wddr_d)
        nc.sync.dma_start(sel8, sel8_d)
        nc.sync.dma_start(wdecp, wdec_d)
        nc.sync.dma_start(b3bc, b3bc_d)

        # ---- encoder: fuseT = relu(Wenc^T v + benc) + catemb-fold ----
        if PHASES >= 2:
            with tc.tile_pool(name="psf", bufs=2, space="PSUM") as psf, \
                 tc.tile_pool(name="psc", bufs=2, space="PSUM") as psc, \
                 tc.tile_pool(name="encs", bufs=2) as encs:
                for m in range(KH):
                    ps = psf.tile([128, R], F32)
                    for ko in range(KF):
                        nc.tensor.matmul(
                            ps, wencp[:, ko, m * 128:(m + 1) * 128],
                            vT[:, ko, :], start=(ko == 0), stop=(ko == KF - 1))
                    pc = psc.tile([128, R], F32)
                    nc.tensor.matmul(pc, catemb[:, m * 128:(m + 1) * 128],
                                     onehot, start=True, stop=True)
                    sc = encs.tile([128, R], F32)
                    nc.scalar.activation(sc, ps, AF.Relu,
                                         bias=benc[:, m:m + 1])
                    nc.vector.tensor_add(fuseT[:, m, :], sc, pc)

        # ---- vv[r] = sum_f v[r,f]^2 (Act engine; emitted after the encoder
        # relus so a late vrow DMA can't stall them on the in-order engine) --
        if PHASES >= 1:
            with tc.tile_pool(name="vvp", bufs=1) as VP, \
                 tc.tile_pool(name="vsq", bufs=2) as SQ:
                vrow = VP.tile([128, MR, F], BF)
                nc.sync.dma_start(vrow, vrow_d)
                for m in range(MR):
                    sq = SQ.tile([128, F], BF)
                    nc.scalar.activation(sq, vrow[:, m, :], AF.Square,
                                         accum_out=vv[:, m:m + 1])
                nc.vector.tensor_scalar_add(vve, vv, 1e-8)

        # ---- xg row-block m: 4 psum groups + x32 bias fold (shared) ----
        def xg_group(psx, m, nb):
            ps = psx.tile([128, 512], F32, tag="x")
            for k in range(KH):
                nc.tensor.matmul(
                    ps, fuseT[:, k, m * 128:(m + 1) * 128],
                    wxp[:, k, nb * 512:(nb + 1) * 512],
                    start=(k == 0), stop=(k == KH - 1))
            # xg carries x32 so it folds into the x32 fp8 Wh partials;
            # bxh128 is host-prescaled by 32.
            def fin():
                nc.vector.scalar_tensor_tensor(
                    xgsb[:, m, nb * 512:(nb + 1) * 512], ps, WH_SCALE,
                    bxh128[:, nb * 512:(nb + 1) * 512], ALU.mult, ALU.add)
            return fin

        # ---- xg ahead of the LSTM (m>0 interleaves into steps if KFILL) ----
        if PHASES >= 3:
            with tc.tile_pool(name="psx0", bufs=2, space="PSUM") as psx0:
                for mm_ in range(1 if KFILL else MR):
                    for nb in range(4):
                        xg_group(psx0, mm_, nb)()

        # ---- LSTM + interleaved distance head + xg m=1..3 + decoder ----
        if PHASES >= 5:
            NBORD = (0, 1, 3, 2)  # i, f, g, o last: c-chain unblocks earlier
            nc.vector.memset(hT.bitcast(DT.uint8), 0)
            nc.vector.memset(hT8.bitcast(DT.uint8), 0)
            with tc.tile_pool(name="psl", bufs=3 if KFILL else 4,
                              space="PSUM") as psl, \
                 tc.tile_pool(name="pstr", bufs=2, space="PSUM") as pstr, \
                 tc.tile_pool(name="psx", bufs=1, space="PSUM") as psx, \
                 tc.tile_pool(name="psq", bufs=1, space="PSUM") as psq, \
                 tc.tile_pool(name="psd", bufs=1, space="PSUM") as psd, \
                 tc.tile_pool(name="gap", bufs=6) as gap, \
                 tc.tile_pool(name="gaop", bufs=4) as gaop, \
                 tc.tile_pool(name="hp", bufs=2) as hp, \
                 tc.tile_pool(name="dfp", bufs=6) as dfp, \
                 tc.tile_pool(name="ltp", bufs=2) as ltp:
                dist_pq = [None] * MR

                def dist_half(m, half):
                    # 8 of the 16 -2*v.p matmuls for row-block m (PE filler)
                    if half == 0:
                        dist_pq[m] = psq.tile([128, 22], F32, tag="q",
                                              name=f"pq{m}")
                    pq = dist_pq[m]
                    for ko in range(8 * half, 8 * half + 8):
                        nc.tensor.matmul(pq, vT[:, ko, m * 128:(m + 1) * 128],
                                         pwT[:, ko, :],
                                         start=(ko == 0), stop=False)
                    if half == 1:
                        nc.tensor.matmul(pq, ones1f, pprow,
                                         start=False, stop=True)

                def dist_finish(m):
                    # dist_feat = log((d+1)/(d+eps)) ~= x - x^2/2, x = 1/d
                    # (d ~ 2700 so the truncation error is ~1e-11); keeps the
                    # act engine on the sigmoid table (no Ln table swap).
                    pq = dist_pq[m]
                    dd = dfp.tile([128, P2], F32)
                    nc.scalar.activation(dd, pq[:, 0:P2], AF.Identity,
                                         bias=vve[:, m:m + 1])
                    nc.scalar.copy(grs[:, m, 0:1], pq[:, P2:P2 + 1])
                    x_ = dfp.tile([128, P2], F32)
                    nc.vector.reciprocal(x_, dd)
                    t_ = dfp.tile([128, P2], F32)
                    nc.vector.tensor_scalar_mul(t_, x_, -0.5)
                    nc.vector.tensor_scalar_add(t_, t_, 1.0)
                    nc.vector.tensor_mul(t_, t_, x_)
                    nc.vector.tensor_mul(t_, t_, wddr[:, m, :])
                    with nc.allow_low_precision(reason="20-wide reduce, f32r"):
                        nc.vector.reduce_sum(out=grs[:, m, 1:2], in_=t_,
                                             axis=AX.X)

                def dist_tail():
                    pr = psd.tile([BC, 2], F32, tag="d")
                    for dm in range(MR):
                        nc.tensor.matmul(pr, sel8, grs[:, dm, :],
                                         start=(dm == 0), stop=(dm == MR - 1))
                    nc.scalar.activation(gsb, pr[:, 0:1], AF.Sigmoid,
                                         bias=b3bc[:, 1:2], scale=1.0 / S)
                    nc.scalar.activation(dsb, pr[:, 1:2], AF.Sigmoid,
                                         bias=b3bc[:, 2:3])

                if not KFILL:
                    for dm in range(MR):
                        dist_half(dm, 0)
                        dist_half(dm, 1)
                        dist_finish(dm)
                    dist_tail()

                OW = 32 if KWH == "mix" else BC

                def emit_folds(s):
                    # xg+bias fold matmuls for step s: independent of h, so
                    # they are emitted one step ahead (into the PE stream
                    # ahead of step s-1's transposes) to fill the PE stall
                    # while act(o)/h of the previous step complete.
                    m = s // 16
                    p0 = (s * 8) % 128
                    blk = (p0 // 64) * 64
                    q = (p0 % 64) // 8
                    pss = {}
                    for nb in NBORD:
                        ps = psl.tile([OW, 512], F32, tag="l", name=f"l{s%4}")
                        nhalf = 2 if (nb == 2 and KOSPLIT) else 1
                        w = 512 // nhalf
                        for hh in range(nhalf):
                            nc.tensor.matmul(
                                ps[:, hh * w:(hh + 1) * w],
                                eyevar[blk:blk + 64, q, 0:OW],
                                xgsb[blk:blk + 64, m,
                                     nb * 512 + hh * w:nb * 512 + (hh + 1) * w],
                                start=True, stop=(s == 0))
                        pss[nb] = ps
                    return pss

                pss_next = emit_folds(0)
                for s in range(S):
                    pss = pss_next
                    ga = {}
                    for nb in NBORD:
                        ps = pss[nb]
                        nhalf = 2 if (nb == 2 and KOSPLIT) else 1
                        w = 512 // nhalf
                        for hh in range(nhalf):
                            if s > 0 and KWH == "mix":
                                # k-tiles 0,1 in one fp8 DoubleRow pass
                                nc.tensor.matmul(
                                    ps[:, hh * w:(hh + 1) * w],
                                    hT8,
                                    wh8sb[:, :, nb * 512 + hh * w:
                                          nb * 512 + (hh + 1) * w],
                                    start=False, stop=False,
                                    perf_mode=mybir.MatmulPerfMode.DoubleRow)
                                for k in (2, 3):
                                    nc.tensor.matmul(
                                        ps[0:BC, hh * w:(hh + 1) * w],
                                        hT[:, k, 0:BC],
                                        whsb[:, k, nb * 512 + hh * w:
                                             nb * 512 + (hh + 1) * w],
                                        start=False, stop=(k == KH - 1),
                                        skip_group_check=True)
                            elif s > 0:
                                for k in range(KH):
                                    nc.tensor.matmul(
                                        ps[0:BC, hh * w:(hh + 1) * w],
                                        hT[:, k, 0:BC],
                                        whsb[:, k, nb * 512 + hh * w:
                                             nb * 512 + (hh + 1) * w],
                                        start=False, stop=(k == KH - 1))
                            gpool = gap if nhalf == 1 else gaop
                            g = gpool.tile([BC, w], BF)
                            nc.scalar.activation(
                                g, ps[0:BC, hh * w:(hh + 1) * w],
                                AF.Tanh if nb == 3 else AF.Sigmoid,
                                scale=1.0 / WH_SCALE)
                            ga.setdefault(nb, []).append(g)
                    if s == 0:
                        nc.vector.tensor_mul(cst, ga[0][0], ga[3][0])
                    else:
                        t1 = ltp.tile([BC, H], BF)
                        nc.vector.tensor_mul(t1, ga[0][0], ga[3][0])  # i*g
                        nc.vector.tensor_mul(cst, cst, ga[1][0])      # f*c
                        nc.vector.tensor_add(cst, cst, t1)
                    if s + 1 < S:
                        pss_next = emit_folds(s + 1)
                    # PE filler between the step's matmuls and transposes:
                    # real work where available, else p-state keepalive.
                    fins = []
                    if KFILL:
                        if s < 12:
                            fins.append(xg_group(psx, 1 + s // 4, s % 4))
                        elif s < 20:
                            dm, dh = (s - 12) // 2, (s - 12) % 2
                            dist_half(dm, dh)
                            if dh == 1:
                                fins.append(lambda dm=dm: dist_finish(dm))
                        else:
                            for _ in range(KDUM):
                                dps = psx.tile([128, 512], F32, tag="x")
                                nc.tensor.matmul(dps, fuseT[:, 0, 0:128],
                                                 wxp[:, 0, 0:512],
                                                 start=True, stop=True)
                    # h = o*c in k-chunks so transpose/cast pipeline per k;
                    # casts alternate DVE/Act to halve the serial tail.
                    h = hp.tile([BC, H], BF)
                    for k in range(KH):
                        if len(ga[2]) == 2:
                            osrc = ga[2][k // 2][:, (k % 2) * 128:
                                                 (k % 2 + 1) * 128]
                        else:
                            osrc = ga[2][0][:, k * 128:(k + 1) * 128]
                        nc.vector.tensor_mul(
                            h[:, k * 128:(k + 1) * 128], osrc,
                            cst[:, k * 128:(k + 1) * 128])
                        pt = pstr.tile([128, BC], BF, tag="tr")
                        nc.tensor.transpose(pt, h[:, k * 128:(k + 1) * 128],
                                            eye8)
                        # all casts on DVE: a cast on the in-order act engine
                        # would block the next step's gate activations
                        if KWH == "mix" and k < 2:
                            nc.vector.tensor_copy(hT8[:, k, 0:BC], pt)
                        else:
                            nc.vector.tensor_copy(hT[:, k, 0:BC], pt)
                    for fcb in fins:
                        fcb()
                    if KFILL and s == 21:
                        dist_tail()
                # decoder
                pd = psd.tile([BC, 2], F32, tag="d")
                for k in range(KH):
                    if KWH == "mix" and k < 2:
                        nc.tensor.matmul(pd[:, 0:1], hT8[:, k, 0:BC],
                                         wdec8p[:, k:k + 1],
                                         start=(k == 0), stop=False)
                    else:
                        nc.tensor.matmul(pd[:, 0:1], hT[:, k, 0:BC],
                                         wdecp[:, k:k + 1],
                                         start=(k == 0), stop=(k == KH - 1))
                nc.scalar.activation(osb, pd[:, 0:1], AF.Sigmoid,
                                     bias=b3bc[:, 0:1], scale=1.0 / WH_SCALE)

        # ---- combine ----
        if PHASES >= 6:
            nc.vector.tensor_sub(fin, osb, dsb)
            nc.vector.scalar_tensor_tensor(fin, fin, gsb[:, 0:1], dsb,
                                           ALU.mult, ALU.add)
            nc.sync.dma_start(out_d, fin)


_NC_CACHE = {}


def _get_nc():
    if "nc" not in _NC_CACHE:
        _NC_CACHE["nc"] = build_nc()
    return _NC_CACHE["nc"]


def _make_in_maps(v_feat, category, W_enc, b_enc, Wx, bx, Wh, bh, cat_emb,
                  W_dec, b_dec, prototype, W_dd, b_dd, W_gate, b_gate):
    f32 = np.float32
    v_feat = np.asarray(v_feat, f32)
    category = np.asarray(category).astype(np.int64)

    wencp = np.ascontiguousarray(
        np.asarray(W_enc, f32).reshape(KF, 128, H).transpose(1, 0, 2)
    ).astype(BF_NP)
    benc = np.ascontiguousarray(
        np.asarray(b_enc, f32).reshape(KH, 128).T).copy()
    catemb = np.asarray(cat_emb, f32).astype(BF_NP)
    wxp = np.ascontiguousarray(
        np.asarray(Wx, f32).reshape(KH, 128, G).transpose(1, 0, 2)
    ).astype(BF_NP)
    bxh128 = np.ascontiguousarray(
        np.tile(WH_SCALE * (np.asarray(bx, f32)
                            + np.asarray(bh, f32)).reshape(1, G),
                (128, 1)))
    whs = (WH_SCALE * np.asarray(Wh, f32)).reshape(KH, 128, G)
    whp8 = np.ascontiguousarray(whs.transpose(1, 0, 2)).astype(BF_NP)
    wh8 = np.ascontiguousarray(whs[0:2].transpose(1, 0, 2)).astype(FP8_NP)
    # eyevar[p, q, j] = 1 iff p%64 == q*8+j (64-aligned step-row selector);
    # cols 8..31 are zero padding so the 32-wide PSUM region is fully started.
    pp_ = np.arange(128)
    eyevar = np.zeros((128, 8, 32), f32)
    for qq in range(8):
        for j in range(BC):
            eyevar[pp_ % 64 == qq * 8 + j, qq, j] = 1.0
    eyevar = eyevar.astype(BF_NP)
    eye8 = np.eye(BC, dtype=f32).astype(BF_NP)
    proto = np.asarray(prototype, f32)
    pw = np.concatenate([-2.0 * proto,
                         np.asarray(W_gate, f32).reshape(1, F),
                         np.zeros((1, F), f32)], axis=0)  # [22, F]
    pwT = np.ascontiguousarray(
        pw.T.reshape(KF, 128, 22).transpose(1, 0, 2)).astype(BF_NP)
    pprow = np.concatenate([(proto * proto).sum(axis=1),
                            np.zeros(2, f32)]).reshape(1, 22).astype(f32)
    ones1 = np.ones((1, 128), f32)
    wdd = np.asarray(W_dd, f32).reshape(S, P2)
    rep = np.repeat(wdd[:, None, :], BC, axis=1).reshape(R, P2)
    wddr = np.ascontiguousarray(rep.reshape(MR, 128, P2).transpose(1, 0, 2))
    sel8 = np.zeros((128, BC), f32)
    sel8[np.arange(128), np.arange(128) % BC] = 1.0
    wdecs = (WH_SCALE * np.asarray(W_dec, f32)).reshape(KH, 128).T
    wdecp8 = np.ascontiguousarray(wdecs).astype(BF_NP)
    wdec8 = np.ascontiguousarray(wdecs[:, 0:2]).astype(FP8_NP)
    b3 = np.array([np.asarray(b_dec, f32).reshape(-1)[0],
                   np.asarray(b_gate, f32).reshape(-1)[0],
                   np.asarray(b_dd, f32).reshape(-1)[0]], f32)
    b3bc = np.ascontiguousarray(np.tile(b3.reshape(1, 3), (BC, 1)))

    common = {
        "wencp": wencp, "benc": benc, "catemb": catemb, "wxp": wxp,
        "bxh128": bxh128, "whp8": whp8, "wh8": wh8, "wdec8": wdec8,
        "eyevar": eyevar, "eye8": eye8,
        "pwT": pwT, "pprow": pprow, "ones1": ones1, "wddr": wddr,
        "sel8": sel8, "wdecp8": wdecp8, "b3bc": b3bc,
    }
    in_maps = []
    for j in range(NCORES):
        vs = np.ascontiguousarray(
            v_feat[j * BC:(j + 1) * BC].transpose(1, 0, 2).reshape(R, F))
        vTn = np.ascontiguousarray(
            vs.reshape(R, KF, 128).transpose(2, 1, 0)).astype(BF_NP)
        vrow = np.ascontiguousarray(
            vs.reshape(MR, 128, F).transpose(1, 0, 2)).astype(BF_NP)
        cats = category[j * BC:(j + 1) * BC]
        onehot = (cats[None, :] == np.arange(3)[:, None]).astype(f32)
        onehot = np.ascontiguousarray(
            np.tile(onehot, (1, S))).astype(BF_NP)  # [3, R], r = s*8+b
        in_maps.append({"vT": vTn, "vrow": vrow, "onehot": onehot, **common})
    return in_maps


def run(trace=False, **inputs):
    nc = _get_nc()
    in_maps = _make_in_maps(**inputs)
    res = run_bass_kernel_spmd(nc, in_maps, list(range(NCORES)), trace=trace)
    out = np.concatenate([res.results[j]["out"] for j in range(NCORES)],
                         axis=0).astype(np.float32)
    return out, res


def kernel(**inputs):
    out, _ = run(trace=False, **inputs)
    return out


# revision 90
# speedup vs baseline: 1.1483x; 1.1159x over previous
"""Trainium2 Bass kernel for nn_ASD_RNN (encoder + fused-gate LSTM + prototype-distance head).

Contract: kernel(**inputs) takes FULL unsharded inputs (as in reference.setup_inputs())
and returns the FULL [64, 1] float32 output. Internally shards batch across 8 cores
(8 batches per core), runs one Bass kernel per core via run_bass_kernel_spmd, gathers.

Per-core layout (BC = 8 batches, R = BC*S = 512 rows, s-major: r = s*8 + b):
  - All GEMM operands are bf16; PSUM accumulation is fp32.
  - v is shipped pre-transposed from the host: vT [f%128, f//128, r] feeds the
    encoder and prototype-distance GEMMs; a row-major copy vrow feeds ||v||^2.
  - Encoder: fuseT[h%128, h//128, r] = relu(Wenc^T v + benc) + cat_emb one-hot fold.
  - xg = fuse @ Wx + (bx+bh), computed row-major into SBUF-resident xgsb
    [r%128, r//128, g] (bias added via a host-replicated [128, G] tensor).
  - LSTM keeps h transposed (hT [h%128, k, b]) as the matmul stationary; Wh is the
    moving operand (one full Wh pass per step is the PE floor). The per-step xg+bias
    contribution is folded into the gates PSUM with a 32-row selector matmul
    (eyevar) so no per-step DMA or staging copies are needed.
  - Distance head: ||v-p||^2 via matmul folds (-2p | W_gate rides as column 20),
    log-ratio via two Ln activations with per-partition bias, reduced via a
    selector matmul.
"""

import numpy as np
import ml_dtypes

import concourse.bass as bass
import concourse.mybir as mybir
import concourse.tile as tile
from concourse import bacc
from concourse.bass_utils import run_bass_kernel_spmd

AF = mybir.ActivationFunctionType
ALU = mybir.AluOpType
DT = mybir.dt
AX = mybir.AxisListType

B, S, F, H, P2 = 64, 64, 2048, 512, 20
G = 4 * H
NCORES = 8
BC = B // NCORES          # 8 batches per core
R = BC * S                # 512 rows per core
KF = F // 128             # 16 k-tiles over F
KH = H // 128             # 4 k-tiles over H
MR = R // 128             # 4 row tiles
F32 = DT.float32
BF = DT.bfloat16
FP8 = DT.float8e4
MMF = DT.float32r
BF_NP = ml_dtypes.bfloat16
FP8_NP = ml_dtypes.float8_e4m3
WH_SCALE = 32.0  # Wh/W_dec stored fp8 as x32; pre-acts carry x32, acts undo

import os as _os
# "bf16" (default): fully unthrottled, fastest measured. "mix" (k-tiles 0-1
# via fp8-DoubleRow) cuts the Wh stream 25% but still trips the PE power
# limiter and measures slower.
KWH = _os.environ.get("KWH", "bf16")
KDUM = int(_os.environ.get("KDUM", "0"))
KOSPLIT = int(_os.environ.get("KOSPLIT", "0"))
KFILL = int(_os.environ.get("KFILL", "0"))


def build_nc():
    nc = bacc.Bacc("TRN2", target_bir_lowering=False, debug=False,
                   num_devices=NCORES)

    def din(name, shape, dt=BF):
        return nc.dram_tensor(name, shape, dt, kind="ExternalInput").ap()

    vT_d = din("vT", [128, KF, R])
    vrow_d = din("vrow", [128, MR, F])
    wenc_d = din("wencp", [128, KF, H])
    benc_d = din("benc", [128, KH], F32)
    catemb_d = din("catemb", [3, H])
    onehot_d = din("onehot", [3, R])
    wx_d = din("wxp", [128, KH, G])
    bxh_d = din("bxh128", [128, G], F32)
    wh_d = din("whp8", [128, KH, G], BF)
    wh8_d = din("wh8", [128, 2, G], FP8)
    wdec8_d = din("wdec8", [128, 2], FP8)
    eyevar_d = din("eyevar", [128, 8, 32])
    eye8_d = din("eye8", [BC, BC])
    pwT_d = din("pwT", [128, KF, 22])
    pprow_d = din("pprow", [1, 22], MMF)
    ones1_d = din("ones1", [1, 128], MMF)
    wddr_d = din("wddr", [128, MR, P2], F32)
    sel8_d = din("sel8", [128, BC], MMF)
    wdec_d = din("wdecp8", [128, KH], BF)
    b3bc_d = din("b3bc", [BC, 3], F32)
    out_d = nc.dram_tensor("out", [BC, 1], F32, kind="ExternalOutput").ap()

    with tile.TileContext(nc) as tc:
        _body(tc, nc, vT_d, vrow_d, wenc_d, benc_d, catemb_d, onehot_d, wx_d,
              bxh_d, wh_d, wh8_d, wdec8_d, eyevar_d, eye8_d, pwT_d, pprow_d,
              ones1_d, wddr_d, sel8_d, wdec_d, b3bc_d, out_d)
    nc.compile()
    return nc


def _body(tc, nc, vT_d, vrow_d, wenc_d, benc_d, catemb_d, onehot_d, wx_d,
          bxh_d, wh_d, wh8_d, wdec8_d, eyevar_d, eye8_d, pwT_d, pprow_d,
          ones1_d, wddr_d, sel8_d, wdec_d, b3bc_d, out_d):
    import os
    PHASES = int(os.environ.get("KPHASES", "9"))
    with tc.tile_pool(name="persist", bufs=1) as P:
        vT = P.tile([128, KF, R], BF)
        wencp = P.tile([128, KF, H], BF)
        wxp = P.tile([128, KH, G], BF)
        whsb = P.tile([128, KH, G], BF)
        if KWH == "mix":
            wh8sb = P.tile([128, 2, G], FP8)
            wdec8p = P.tile([128, 2], FP8)
        fuseT = P.tile([128, KH, R], BF)
        xgsb = P.tile([128, MR, G], BF)
        bxh128 = P.tile([128, G], F32)
        catemb = P.tile([3, H], BF)
        onehot = P.tile([3, R], BF)
        benc = P.tile([128, KH], F32)
        pwT = P.tile([128, KF, 22], BF)
        pprow = P.tile([1, 22], MMF)
        ones1f = P.tile([1, 128], MMF)
        wddr = P.tile([128, MR, P2], F32)
        sel8 = P.tile([128, BC], MMF)
        eyevar = P.tile([128, 8, 32], BF)
        eye8 = P.tile([BC, BC], BF)
        wdecp = P.tile([128, KH], BF)
        b3bc = P.tile([BC, 3], F32)
        hT = P.tile([128, KH, BC], BF)
        if KWH == "mix":
            # 32-wide padding: DoubleRow needs >=32 stationary cols
            hT8 = P.tile([128, 2, 32], FP8)
        cst = P.tile([BC, H], BF)
        vv = P.tile([128, MR], F32)
        vve = P.tile([128, MR], F32)
        grs = P.tile([128, MR, 2], MMF)
        osb = P.tile([BC, 1], F32)
        gsb = P.tile([BC, 1], F32)
        dsb = P.tile([BC, 1], F32)
        fin = P.tile([BC, 1], F32)

        # ---- input DMAs (queue order = priority: encoder set first; vT and
        # wencp chunked by ko so the encoder k-loop can start early) ----
        for c in range(4):
            nc.sync.dma_start(vT[:, 4 * c:4 * (c + 1), :],
                              vT_d[:, 4 * c:4 * (c + 1), :])
        for c in range(2):
            nc.sync.dma_start(wencp[:, 8 * c:8 * (c + 1), :],
                              wenc_d[:, 8 * c:8 * (c + 1), :])
        nc.sync.dma_start(benc, benc_d)
        nc.sync.dma_start(catemb, catemb_d)
        nc.sync.dma_start(onehot, onehot_d)
        nc.sync.dma_start(wxp, wx_d)
        nc.sync.dma_start(bxh128, bxh_d)
        nc.sync.dma_start(whsb, wh_d)
        if KWH == "mix":
            nc.sync.dma_start(wh8sb, wh8_d)
            nc.sync.dma_start(wdec8p, wdec8_d)
        nc.sync.dma_start(eyevar, eyevar_d)
        nc.sync.dma_start(eye8, eye8_d)
        nc.sync.dma_start(pwT, pwT_d)
        nc.sync.dma_start(pprow, pprow_d)
        nc.sync.dma_start(ones1f, ones1_d)
        nc.sync.dma_start(wddr, wddr_d)
        nc.sync.dma_start(sel8, sel8_d)
        nc.sync.dma_start(wdecp, wdec_d)
        nc.sync.dma_start(b3bc, b3bc_d)

        # ---- encoder: fuseT = relu(Wenc^T v + benc) + catemb-fold ----
        if PHASES >= 2:
            with tc.tile_pool(name="psf", bufs=2, space="PSUM") as psf, \
                 tc.tile_pool(name="psc", bufs=2, space="PSUM") as psc, \
                 tc.tile_pool(name="encs", bufs=2) as encs:
                for m in range(KH):
                    ps = psf.tile([128, R], F32)
                    for ko in range(KF):
                        nc.tensor.matmul(
                            ps, wencp[:, ko, m * 128:(m + 1) * 128],
                            vT[:, ko, :], start=(ko == 0), stop=(ko == KF - 1))
                    pc = psc.tile([128, R], F32)
                    nc.tensor.matmul(pc, catemb[:, m * 128:(m + 1) * 128],
                                     onehot, start=True, stop=True)
                    sc = encs.tile([128, R], F32)
                    nc.scalar.activation(sc, ps, AF.Relu,
                                         bias=benc[:, m:m + 1])
                    nc.vector.tensor_add(fuseT[:, m, :], sc, pc)

        # ---- vv[r] = sum_f v[r,f]^2 (Act engine; emitted after the encoder
        # relus so a late vrow DMA can't stall them on the in-order engine) --
        if PHASES >= 1:
            with tc.tile_pool(name="vvp", bufs=1) as VP, \
                 tc.tile_pool(name="vsq", bufs=2) as SQ:
                vrow = VP.tile([128, MR, F], BF)
                nc.sync.dma_start(vrow, vrow_d)
                for m in range(MR):
                    sq = SQ.tile([128, F], BF)
                    nc.scalar.activation(sq, vrow[:, m, :], AF.Square,
                                         accum_out=vv[:, m:m + 1])
                nc.vector.tensor_scalar_add(vve, vv, 1e-8)

        # ---- xg row-block m: 4 psum groups + x32 bias fold (shared) ----
        def xg_group(psx, m, nb):
            ps = psx.tile([128, 512], F32, tag="x")
            for k in range(KH):
                nc.tensor.matmul(
                    ps, fuseT[:, k, m * 128:(m + 1) * 128],
                    wxp[:, k, nb * 512:(nb + 1) * 512],
                    start=(k == 0), stop=(k == KH - 1))
            # xg carries x32 so it folds into the x32 fp8 Wh partials;
            # bxh128 is host-prescaled by 32.
            def fin():
                nc.vector.scalar_tensor_tensor(
                    xgsb[:, m, nb * 512:(nb + 1) * 512], ps, WH_SCALE,
                    bxh128[:, nb * 512:(nb + 1) * 512], ALU.mult, ALU.add)
            return fin

        # ---- xg ahead of the LSTM (m>0 interleaves into steps if KFILL) ----
        if PHASES >= 3:
            with tc.tile_pool(name="psx0", bufs=2, space="PSUM") as psx0:
                for mm_ in range(1 if KFILL else MR):
                    for nb in range(4):
                        xg_group(psx0, mm_, nb)()

        # ---- LSTM + interleaved distance head + xg m=1..3 + decoder ----
        if PHASES >= 5:
            NBORD = (0, 1, 3, 2)  # i, f, g, o last: c-chain unblocks earlier
            nc.vector.memset(hT.bitcast(DT.uint8), 0)
            if KWH == "mix":
                nc.vector.memset(hT8.bitcast(DT.uint8), 0)
            with tc.tile_pool(name="psl", bufs=3 if KFILL else 4,
                              space="PSUM") as psl, \
                 tc.tile_pool(name="pstr", bufs=2, space="PSUM") as pstr, \
                 tc.tile_pool(name="psx", bufs=1, space="PSUM") as psx, \
                 tc.tile_pool(name="psq", bufs=1, space="PSUM") as psq, \
                 tc.tile_pool(name="psd", bufs=1, space="PSUM") as psd, \
                 tc.tile_pool(name="gap", bufs=6) as gap, \
                 tc.tile_pool(name="gaop", bufs=4) as gaop, \
                 tc.tile_pool(name="hp", bufs=2) as hp, \
                 tc.tile_pool(name="dfp", bufs=6) as dfp, \
                 tc.tile_pool(name="ltp", bufs=2) as ltp:
                dist_pq = [None] * MR

                def dist_half(m, half):
                    # 8 of the 16 -2*v.p matmuls for row-block m (PE filler)
                    if half == 0:
                        dist_pq[m] = psq.tile([128, 22], F32, tag="q",
                                              name=f"pq{m}")
                    pq = dist_pq[m]
                    for ko in range(8 * half, 8 * half + 8):
                        nc.tensor.matmul(pq, vT[:, ko, m * 128:(m + 1) * 128],
                                         pwT[:, ko, :],
                                         start=(ko == 0), stop=False)
                    if half == 1:
                        nc.tensor.matmul(pq, ones1f, pprow,
                                         start=False, stop=True)

                def dist_finish(m):
                    # dist_feat = log((d+1)/(d+eps)) ~= x - x^2/2, x = 1/d
                    # (d ~ 2700 so the truncation error is ~1e-11); keeps the
                    # act engine on the sigmoid table (no Ln table swap).
                    pq = dist_pq[m]
                    dd = dfp.tile([128, P2], F32)
                    nc.scalar.activation(dd, pq[:, 0:P2], AF.Identity,
                                         bias=vve[:, m:m + 1])
                    nc.scalar.copy(grs[:, m, 0:1], pq[:, P2:P2 + 1])
                    x_ = dfp.tile([128, P2], F32)
                    nc.vector.reciprocal(x_, dd)
                    t_ = dfp.tile([128, P2], F32)
                    nc.vector.tensor_scalar_mul(t_, x_, -0.5)
                    nc.vector.tensor_scalar_add(t_, t_, 1.0)
                    nc.vector.tensor_mul(t_, t_, x_)
                    nc.vector.tensor_mul(t_, t_, wddr[:, m, :])
                    with nc.allow_low_precision(reason="20-wide reduce, f32r"):
                        nc.vector.reduce_sum(out=grs[:, m, 1:2], in_=t_,
                                             axis=AX.X)

                def dist_tail():
                    pr = psd.tile([BC, 2], F32, tag="d")
                    for dm in range(MR):
                        nc.tensor.matmul(pr, sel8, grs[:, dm, :],
                                         start=(dm == 0), stop=(dm == MR - 1))
                    nc.scalar.activation(gsb, pr[:, 0:1], AF.Sigmoid,
                                         bias=b3bc[:, 1:2], scale=1.0 / S)
                    nc.scalar.activation(dsb, pr[:, 1:2], AF.Sigmoid,
                                         bias=b3bc[:, 2:3])

                if not KFILL:
                    for dm in range(MR):
                        dist_half(dm, 0)
                        dist_half(dm, 1)
                        dist_finish(dm)
                    dist_tail()

                OW = 32 if KWH == "mix" else BC

                def emit_folds(s):
                    # xg+bias fold matmuls for step s: independent of h, so
                    # they are emitted one step ahead (into the PE stream
                    # ahead of step s-1's transposes) to fill the PE stall
                    # while act(o)/h of the previous step complete.
                    m = s // 16
                    p0 = (s * 8) % 128
                    blk = (p0 // 64) * 64
                    q = (p0 % 64) // 8
                    pss = {}
                    for nb in NBORD:
                        ps = psl.tile([OW, 512], F32, tag="l", name=f"l{s%4}")
                        nhalf = 2 if (nb == 2 and KOSPLIT) else 1
                        w = 512 // nhalf
                        for hh in range(nhalf):
                            nc.tensor.matmul(
                                ps[:, hh * w:(hh + 1) * w],
                                eyevar[blk:blk + 64, q, 0:OW],
                                xgsb[blk:blk + 64, m,
                                     nb * 512 + hh * w:nb * 512 + (hh + 1) * w],
                                start=True, stop=(s == 0))
                        pss[nb] = ps
                    return pss

                pss_next = emit_folds(0)
                for s in range(S):
                    pss = pss_next
                    ga = {}
                    for nb in NBORD:
                        ps = pss[nb]
                        nhalf = 2 if (nb == 2 and KOSPLIT) else 1
                        w = 512 // nhalf
                        for hh in range(nhalf):
                            if s > 0 and KWH == "mix":
                                # k-tiles 0,1 in one fp8 DoubleRow pass
                                nc.tensor.matmul(
                                    ps[:, hh * w:(hh + 1) * w],
                                    hT8,
                                    wh8sb[:, :, nb * 512 + hh * w:
                                          nb * 512 + (hh + 1) * w],
                                    start=False, stop=False,
                                    perf_mode=mybir.MatmulPerfMode.DoubleRow)
                                for k in (2, 3):
                                    nc.tensor.matmul(
                                        ps[0:BC, hh * w:(hh + 1) * w],
                                        hT[:, k, 0:BC],
                                        whsb[:, k, nb * 512 + hh * w:
                                             nb * 512 + (hh + 1) * w],
                                        start=False, stop=(k == KH - 1),
                                        skip_group_check=True)
                            elif s > 0:
                                for k in range(KH):
                                    nc.tensor.matmul(
                                        ps[0:BC, hh * w:(hh + 1) * w],
                                        hT[:, k, 0:BC],
                                        whsb[:, k, nb * 512 + hh * w:
                                             nb * 512 + (hh + 1) * w],
                                        start=False, stop=(k == KH - 1))
                            gpool = gap if nhalf == 1 else gaop
                            g = gpool.tile([BC, w], BF)
                            nc.scalar.activation(
                                g, ps[0:BC, hh * w:(hh + 1) * w],
                                AF.Tanh if nb == 3 else AF.Sigmoid,
                                scale=1.0 / WH_SCALE)
                            ga.setdefault(nb, []).append(g)
                    if s == 0:
                        nc.vector.tensor_mul(cst, ga[0][0], ga[3][0])
                    else:
                        t1 = ltp.tile([BC, H], BF)
                        nc.vector.tensor_mul(t1, ga[0][0], ga[3][0])  # i*g
                        nc.vector.tensor_mul(cst, cst, ga[1][0])      # f*c
                        nc.vector.tensor_add(cst, cst, t1)
                    if s + 1 < S:
                        pss_next = emit_folds(s + 1)
                    # PE filler between the step's matmuls and transposes:
                    # real work where available, else p-state keepalive.
                    fins = []
                    if KFILL:
                        if s < 12:
                            fins.append(xg_group(psx, 1 + s // 4, s % 4))
                        elif s < 20:
                            dm, dh = (s - 12) // 2, (s - 12) % 2
                            dist_half(dm, dh)
                            if dh == 1:
                                fins.append(lambda dm=dm: dist_finish(dm))
                        else:
                            for _ in range(KDUM):
                                dps = psx.tile([128, 512], F32, tag="x")
                                nc.tensor.matmul(dps, fuseT[:, 0, 0:128],
                                                 wxp[:, 0, 0:512],
                                                 start=True, stop=True)
                    # h = o*c in k-chunks so transpose/cast pipeline per k;
                    # casts alternate DVE/Act to halve the serial tail.
                    h = hp.tile([BC, H], BF)
                    for k in range(KH):
                        if len(ga[2]) == 2:
                            osrc = ga[2][k // 2][:, (k % 2) * 128:
                                                 (k % 2 + 1) * 128]
                        else:
                            osrc = ga[2][0][:, k * 128:(k + 1) * 128]
                        nc.vector.tensor_mul(
                            h[:, k * 128:(k + 1) * 128], osrc,
                            cst[:, k * 128:(k + 1) * 128])
                        pt = pstr.tile([128, BC], BF, tag="tr")
                        nc.tensor.transpose(pt, h[:, k * 128:(k + 1) * 128],
                                            eye8)
                        # all casts on DVE: a cast on the in-order act engine
                        # would block the next step's gate activations
                        if KWH == "mix" and k < 2:
                            nc.vector.tensor_copy(hT8[:, k, 0:BC], pt)
                        else:
                            nc.vector.tensor_copy(hT[:, k, 0:BC], pt)
                    for fcb in fins:
                        fcb()
                    if KFILL and s == 21:
                        dist_tail()
                # decoder
                pd = psd.tile([BC, 2], F32, tag="d")
                for k in range(KH):
                    if KWH == "mix" and k < 2:
                        nc.tensor.matmul(pd[:, 0:1], hT8[:, k, 0:BC],
                                         wdec8p[:, k:k + 1],
                                         start=(k == 0), stop=False)
                    else:
                        nc.tensor.matmul(pd[:, 0:1], hT[:, k, 0:BC],
                                         wdecp[:, k:k + 1],
                                         start=(k == 0), stop=(k == KH - 1))
                nc.scalar.activation(osb, pd[:, 0:1], AF.Sigmoid,
                                     bias=b3bc[:, 0:1], scale=1.0 / WH_SCALE)

        # ---- combine ----
        if PHASES >= 6:
            nc.vector.tensor_sub(fin, osb, dsb)
            nc.vector.scalar_tensor_tensor(fin, fin, gsb[:, 0:1], dsb,
                                           ALU.mult, ALU.add)
            nc.sync.dma_start(out_d, fin)


_NC_CACHE = {}


def _get_nc():
    if "nc" not in _NC_CACHE:
        _NC_CACHE["nc"] = build_nc()
    return _NC_CACHE["nc"]


def _make_in_maps(v_feat, category, W_enc, b_enc, Wx, bx, Wh, bh, cat_emb,
                  W_dec, b_dec, prototype, W_dd, b_dd, W_gate, b_gate):
    f32 = np.float32
    v_feat = np.asarray(v_feat, f32)
    category = np.asarray(category).astype(np.int64)

    wencp = np.ascontiguousarray(
        np.asarray(W_enc, f32).reshape(KF, 128, H).transpose(1, 0, 2)
    ).astype(BF_NP)
    benc = np.ascontiguousarray(
        np.asarray(b_enc, f32).reshape(KH, 128).T).copy()
    catemb = np.asarray(cat_emb, f32).astype(BF_NP)
    wxp = np.ascontiguousarray(
        np.asarray(Wx, f32).reshape(KH, 128, G).transpose(1, 0, 2)
    ).astype(BF_NP)
    bxh128 = np.ascontiguousarray(
        np.tile(WH_SCALE * (np.asarray(bx, f32)
                            + np.asarray(bh, f32)).reshape(1, G),
                (128, 1)))
    whs = (WH_SCALE * np.asarray(Wh, f32)).reshape(KH, 128, G)
    whp8 = np.ascontiguousarray(whs.transpose(1, 0, 2)).astype(BF_NP)
    wh8 = np.ascontiguousarray(whs[0:2].transpose(1, 0, 2)).astype(FP8_NP)
    # eyevar[p, q, j] = 1 iff p%64 == q*8+j (64-aligned step-row selector);
    # cols 8..31 are zero padding so the 32-wide PSUM region is fully started.
    pp_ = np.arange(128)
    eyevar = np.zeros((128, 8, 32), f32)
    for qq in range(8):
        for j in range(BC):
            eyevar[pp_ % 64 == qq * 8 + j, qq, j] = 1.0
    eyevar = eyevar.astype(BF_NP)
    eye8 = np.eye(BC, dtype=f32).astype(BF_NP)
    proto = np.asarray(prototype, f32)
    pw = np.concatenate([-2.0 * proto,
                         np.asarray(W_gate, f32).reshape(1, F),
                         np.zeros((1, F), f32)], axis=0)  # [22, F]
    pwT = np.ascontiguousarray(
        pw.T.reshape(KF, 128, 22).transpose(1, 0, 2)).astype(BF_NP)
    pprow = np.concatenate([(proto * proto).sum(axis=1),
                            np.zeros(2, f32)]).reshape(1, 22).astype(f32)
    ones1 = np.ones((1, 128), f32)
    wdd = np.asarray(W_dd, f32).reshape(S, P2)
    rep = np.repeat(wdd[:, None, :], BC, axis=1).reshape(R, P2)
    wddr = np.ascontiguousarray(rep.reshape(MR, 128, P2).transpose(1, 0, 2))
    sel8 = np.zeros((128, BC), f32)
    sel8[np.arange(128), np.arange(128) % BC] = 1.0
    wdecs = (WH_SCALE * np.asarray(W_dec, f32)).reshape(KH, 128).T
    wdecp8 = np.ascontiguousarray(wdecs).astype(BF_NP)
    wdec8 = np.ascontiguousarray(wdecs[:, 0:2]).astype(FP8_NP)
    b3 = np.array([np.asarray(b_dec, f32).reshape(-1)[0],
                   np.asarray(b_gate, f32).reshape(-1)[0],
                   np.asarray(b_dd, f32).reshape(-1)[0]], f32)
    b3bc = np.ascontiguousarray(np.tile(b3.reshape(1, 3), (BC, 1)))

    common = {
        "wencp": wencp, "benc": benc, "catemb": catemb, "wxp": wxp,
        "bxh128": bxh128, "whp8": whp8, "wh8": wh8, "wdec8": wdec8,
        "eyevar": eyevar, "eye8": eye8,
        "pwT": pwT, "pprow": pprow, "ones1": ones1, "wddr": wddr,
        "sel8": sel8, "wdecp8": wdecp8, "b3bc": b3bc,
    }
    in_maps = []
    for j in range(NCORES):
        vs = np.ascontiguousarray(
            v_feat[j * BC:(j + 1) * BC].transpose(1, 0, 2).reshape(R, F))
        vTn = np.ascontiguousarray(
            vs.reshape(R, KF, 128).transpose(2, 1, 0)).astype(BF_NP)
        vrow = np.ascontiguousarray(
            vs.reshape(MR, 128, F).transpose(1, 0, 2)).astype(BF_NP)
        cats = category[j * BC:(j + 1) * BC]
        onehot = (cats[None, :] == np.arange(3)[:, None]).astype(f32)
        onehot = np.ascontiguousarray(
            np.tile(onehot, (1, S))).astype(BF_NP)  # [3, R], r = s*8+b
        in_maps.append({"vT": vTn, "vrow": vrow, "onehot": onehot, **common})
    return in_maps


def run(trace=False, **inputs):
    nc = _get_nc()
    in_maps = _make_in_maps(**inputs)
    res = run_bass_kernel_spmd(nc, in_maps, list(range(NCORES)), trace=trace)
    out = np.concatenate([res.results[j]["out"] for j in range(NCORES)],
                         axis=0).astype(np.float32)
    return out, res


def kernel(**inputs):
    out, _ = run(trace=False, **inputs)
    return out


# revision 91
# speedup vs baseline: 1.2202x; 1.0627x over previous
"""Trainium2 Bass kernel for nn_ASD_RNN (encoder + fused-gate LSTM + prototype-distance head).

Contract: kernel(**inputs) takes FULL unsharded inputs (as in reference.setup_inputs())
and returns the FULL [64, 1] float32 output. Internally shards batch across 8 cores
(8 batches per core), runs one Bass kernel per core via run_bass_kernel_spmd, gathers.

Per-core layout (BC = 8 batches, R = BC*S = 512 rows, s-major: r = s*8 + b):
  - All GEMM operands are bf16; PSUM accumulation is fp32.
  - v is shipped pre-transposed from the host: vT [f%128, f//128, r] feeds the
    encoder and prototype-distance GEMMs; a row-major copy vrow feeds ||v||^2.
  - Encoder: fuseT[h%128, h//128, r] = relu(Wenc^T v + benc) + cat_emb one-hot fold.
  - xg = fuse @ Wx + (bx+bh), computed row-major into SBUF-resident xgsb
    [r%128, r//128, g] (bias added via a host-replicated [128, G] tensor).
  - LSTM keeps h transposed (hT [h%128, k, b]) as the matmul stationary; Wh is the
    moving operand (one full Wh pass per step is the PE floor). The per-step xg+bias
    contribution is folded into the gates PSUM with a 32-row selector matmul
    (eyevar) so no per-step DMA or staging copies are needed.
  - Distance head: ||v-p||^2 via matmul folds (-2p | W_gate rides as column 20),
    log-ratio via two Ln activations with per-partition bias, reduced via a
    selector matmul.
"""

import numpy as np
import ml_dtypes

import concourse.bass as bass
import concourse.mybir as mybir
import concourse.tile as tile
from concourse import bacc
from concourse.bass_utils import run_bass_kernel_spmd

AF = mybir.ActivationFunctionType
ALU = mybir.AluOpType
DT = mybir.dt
AX = mybir.AxisListType

B, S, F, H, P2 = 64, 64, 2048, 512, 20
G = 4 * H
NCORES = 8
BC = B // NCORES          # 8 batches per core
R = BC * S                # 512 rows per core
KF = F // 128             # 16 k-tiles over F
KH = H // 128             # 4 k-tiles over H
MR = R // 128             # 4 row tiles
F32 = DT.float32
BF = DT.bfloat16
FP8 = DT.float8e4
MMF = DT.float32r
BF_NP = ml_dtypes.bfloat16
FP8_NP = ml_dtypes.float8_e4m3
WH_SCALE = 32.0  # Wh/W_dec stored fp8 as x32; pre-acts carry x32, acts undo

import os as _os
# "bf16" (default): fully unthrottled, fastest measured. "mix" (k-tiles 0-1
# via fp8-DoubleRow) cuts the Wh stream 25% but still trips the PE power
# limiter and measures slower.
KWH = _os.environ.get("KWH", "bf16")
KDUM = int(_os.environ.get("KDUM", "0"))
KOSPLIT = int(_os.environ.get("KOSPLIT", "0"))
KFILL = int(_os.environ.get("KFILL", "0"))


def build_nc():
    nc = bacc.Bacc("TRN2", target_bir_lowering=False, debug=False,
                   num_devices=NCORES)

    def din(name, shape, dt=BF):
        return nc.dram_tensor(name, shape, dt, kind="ExternalInput").ap()

    vT_d = din("vT", [128, KF, R])
    vrow_d = din("vrow", [128, MR, F])
    wenc_d = din("wencp", [128, KF, H])
    benc_d = din("benc", [128, KH], F32)
    catemb_d = din("catemb", [3, H])
    onehot_d = din("onehot", [3, R])
    wx_d = din("wxp", [128, KH, G])
    bxh_d = din("bxh128", [128, G], F32)
    wh_d = din("whp8", [128, KH, G], BF)
    wh8_d = din("wh8", [128, 2, G], FP8)
    wdec8_d = din("wdec8", [128, 2], FP8)
    eyevar_d = din("eyevar", [128, 8, 32])
    eye8_d = din("eye8", [BC, BC])
    pwT_d = din("pwT", [128, KF, 22])
    pprow_d = din("pprow", [1, 22], MMF)
    ones1_d = din("ones1", [1, 128], MMF)
    wddr_d = din("wddr", [128, MR, P2], F32)
    sel8_d = din("sel8", [128, BC], MMF)
    wdec_d = din("wdecp8", [128, KH], BF)
    b3bc_d = din("b3bc", [BC, 3], F32)
    out_d = nc.dram_tensor("out", [BC, 1], F32, kind="ExternalOutput").ap()

    with tile.TileContext(nc) as tc:
        _body(tc, nc, vT_d, vrow_d, wenc_d, benc_d, catemb_d, onehot_d, wx_d,
              bxh_d, wh_d, wh8_d, wdec8_d, eyevar_d, eye8_d, pwT_d, pprow_d,
              ones1_d, wddr_d, sel8_d, wdec_d, b3bc_d, out_d)
    nc.compile()
    return nc


def _body(tc, nc, vT_d, vrow_d, wenc_d, benc_d, catemb_d, onehot_d, wx_d,
          bxh_d, wh_d, wh8_d, wdec8_d, eyevar_d, eye8_d, pwT_d, pprow_d,
          ones1_d, wddr_d, sel8_d, wdec_d, b3bc_d, out_d):
    import os
    PHASES = int(os.environ.get("KPHASES", "9"))
    with tc.tile_pool(name="persist", bufs=1) as P:
        # whsb first: the Wh stream is the hottest SBUF reader (3.5us/step);
        # placement measurably shifts bank-conflict behavior.
        whsb = P.tile([128, KH, G], BF)
        vT = P.tile([128, KF, R], BF)
        wencp = P.tile([128, KF, H], BF)
        wxp = P.tile([128, KH, G], BF)
        if KWH == "mix":
            wh8sb = P.tile([128, 2, G], FP8)
            wdec8p = P.tile([128, 2], FP8)
        fuseT = P.tile([128, KH, R], BF)
        xgsb = P.tile([128, MR, G], BF)
        bxh128 = P.tile([128, G], F32)
        catemb = P.tile([3, H], BF)
        onehot = P.tile([3, R], BF)
        benc = P.tile([128, KH], F32)
        pwT = P.tile([128, KF, 22], BF)
        pprow = P.tile([1, 22], MMF)
        ones1f = P.tile([1, 128], MMF)
        wddr = P.tile([128, MR, P2], F32)
        sel8 = P.tile([128, BC], MMF)
        eyevar = P.tile([128, 8, 32], BF)
        eye8 = P.tile([BC, BC], BF)
        wdecp = P.tile([128, KH], BF)
        b3bc = P.tile([BC, 3], F32)
        hT = P.tile([128, KH, BC], BF)
        if KWH == "mix":
            # 32-wide padding: DoubleRow needs >=32 stationary cols
            hT8 = P.tile([128, 2, 32], FP8)
        cst = P.tile([BC, H], BF)
        vv = P.tile([128, MR], F32)
        vve = P.tile([128, MR], F32)
        grs = P.tile([128, MR, 2], MMF)
        osb = P.tile([BC, 1], F32)
        gsb = P.tile([BC, 1], F32)
        dsb = P.tile([BC, 1], F32)
        fin = P.tile([BC, 1], F32)

        # ---- input DMAs (queue order = priority: encoder set first; vT and
        # wencp chunked by ko so the encoder k-loop can start early) ----
        for c in range(4):
            nc.sync.dma_start(vT[:, 4 * c:4 * (c + 1), :],
                              vT_d[:, 4 * c:4 * (c + 1), :])
        for c in range(2):
            nc.sync.dma_start(wencp[:, 8 * c:8 * (c + 1), :],
                              wenc_d[:, 8 * c:8 * (c + 1), :])
        nc.sync.dma_start(benc, benc_d)
        nc.sync.dma_start(catemb, catemb_d)
        nc.sync.dma_start(onehot, onehot_d)
        nc.sync.dma_start(wxp, wx_d)
        nc.sync.dma_start(bxh128, bxh_d)
        nc.sync.dma_start(whsb, wh_d)
        if KWH == "mix":
            nc.sync.dma_start(wh8sb, wh8_d)
            nc.sync.dma_start(wdec8p, wdec8_d)
        nc.sync.dma_start(eyevar, eyevar_d)
        nc.sync.dma_start(eye8, eye8_d)
        nc.sync.dma_start(pwT, pwT_d)
        nc.sync.dma_start(pprow, pprow_d)
        nc.sync.dma_start(ones1f, ones1_d)
        nc.sync.dma_start(wddr, wddr_d)
        nc.sync.dma_start(sel8, sel8_d)
        nc.sync.dma_start(wdecp, wdec_d)
        nc.sync.dma_start(b3bc, b3bc_d)

        # ---- encoder: fuseT = relu(Wenc^T v + benc) + catemb-fold ----
        if PHASES >= 2:
            with tc.tile_pool(name="psf", bufs=2, space="PSUM") as psf, \
                 tc.tile_pool(name="psc", bufs=2, space="PSUM") as psc, \
                 tc.tile_pool(name="encs", bufs=2) as encs:
                for m in range(KH):
                    ps = psf.tile([128, R], F32)
                    for ko in range(KF):
                        nc.tensor.matmul(
                            ps, wencp[:, ko, m * 128:(m + 1) * 128],
                            vT[:, ko, :], start=(ko == 0), stop=(ko == KF - 1))
                    pc = psc.tile([128, R], F32)
                    nc.tensor.matmul(pc, catemb[:, m * 128:(m + 1) * 128],
                                     onehot, start=True, stop=True)
                    sc = encs.tile([128, R], F32)
                    nc.scalar.activation(sc, ps, AF.Relu,
                                         bias=benc[:, m:m + 1])
                    nc.vector.tensor_add(fuseT[:, m, :], sc, pc)

        # ---- vv[r] = sum_f v[r,f]^2 (Act engine; emitted after the encoder
        # relus so a late vrow DMA can't stall them on the in-order engine) --
        if PHASES >= 1:
            with tc.tile_pool(name="vvp", bufs=1) as VP, \
                 tc.tile_pool(name="vsq", bufs=2) as SQ:
                vrow = VP.tile([128, MR, F], BF)
                nc.sync.dma_start(vrow, vrow_d)
                for m in range(MR):
                    sq = SQ.tile([128, F], BF)
                    nc.scalar.activation(sq, vrow[:, m, :], AF.Square,
                                         accum_out=vv[:, m:m + 1])
                nc.vector.tensor_scalar_add(vve, vv, 1e-8)

        # ---- xg row-block m: 4 psum groups + x32 bias fold (shared) ----
        def xg_group(psx, m, nb):
            ps = psx.tile([128, 512], F32, tag="x")
            for k in range(KH):
                nc.tensor.matmul(
                    ps, fuseT[:, k, m * 128:(m + 1) * 128],
                    wxp[:, k, nb * 512:(nb + 1) * 512],
                    start=(k == 0), stop=(k == KH - 1))
            # xg carries x32 so it folds into the x32 fp8 Wh partials;
            # bxh128 is host-prescaled by 32.
            def fin():
                nc.vector.scalar_tensor_tensor(
                    xgsb[:, m, nb * 512:(nb + 1) * 512], ps, WH_SCALE,
                    bxh128[:, nb * 512:(nb + 1) * 512], ALU.mult, ALU.add)
            return fin

        # ---- xg ahead of the LSTM (m>0 interleaves into steps if KFILL) ----
        if PHASES >= 3:
            with tc.tile_pool(name="psx0", bufs=2, space="PSUM") as psx0:
                for mm_ in range(1 if KFILL else MR):
                    for nb in range(4):
                        xg_group(psx0, mm_, nb)()

        # ---- LSTM + interleaved distance head + xg m=1..3 + decoder ----
        if PHASES >= 5:
            NBORD = (0, 1, 3, 2)  # i, f, g, o last: c-chain unblocks earlier
            nc.vector.memset(hT.bitcast(DT.uint8), 0)
            if KWH == "mix":
                nc.vector.memset(hT8.bitcast(DT.uint8), 0)
            with tc.tile_pool(name="psl", bufs=3 if KFILL else 4,
                              space="PSUM") as psl, \
                 tc.tile_pool(name="pstr", bufs=2, space="PSUM") as pstr, \
                 tc.tile_pool(name="psx", bufs=1, space="PSUM") as psx, \
                 tc.tile_pool(name="psq", bufs=1, space="PSUM") as psq, \
                 tc.tile_pool(name="psd", bufs=1, space="PSUM") as psd, \
                 tc.tile_pool(name="gap", bufs=6) as gap, \
                 tc.tile_pool(name="gaop", bufs=4) as gaop, \
                 tc.tile_pool(name="hp", bufs=2) as hp, \
                 tc.tile_pool(name="dfp", bufs=6) as dfp, \
                 tc.tile_pool(name="ltp", bufs=2) as ltp:
                dist_pq = [None] * MR

                def dist_half(m, half):
                    # 8 of the 16 -2*v.p matmuls for row-block m (PE filler)
                    if half == 0:
                        dist_pq[m] = psq.tile([128, 22], F32, tag="q",
                                              name=f"pq{m}")
                    pq = dist_pq[m]
                    for ko in range(8 * half, 8 * half + 8):
                        nc.tensor.matmul(pq, vT[:, ko, m * 128:(m + 1) * 128],
                                         pwT[:, ko, :],
                                         start=(ko == 0), stop=False)
                    if half == 1:
                        nc.tensor.matmul(pq, ones1f, pprow,
                                         start=False, stop=True)

                def dist_finish(m):
                    # dist_feat = log((d+1)/(d+eps)) ~= x - x^2/2, x = 1/d
                    # (d ~ 2700 so the truncation error is ~1e-11); keeps the
                    # act engine on the sigmoid table (no Ln table swap).
                    pq = dist_pq[m]
                    dd = dfp.tile([128, P2], F32)
                    nc.scalar.activation(dd, pq[:, 0:P2], AF.Identity,
                                         bias=vve[:, m:m + 1])
                    nc.scalar.copy(grs[:, m, 0:1], pq[:, P2:P2 + 1])
                    x_ = dfp.tile([128, P2], F32)
                    nc.vector.reciprocal(x_, dd)
                    t_ = dfp.tile([128, P2], F32)
                    nc.vector.tensor_scalar_mul(t_, x_, -0.5)
                    nc.vector.tensor_scalar_add(t_, t_, 1.0)
                    nc.vector.tensor_mul(t_, t_, x_)
                    nc.vector.tensor_mul(t_, t_, wddr[:, m, :])
                    with nc.allow_low_precision(reason="20-wide reduce, f32r"):
                        nc.vector.reduce_sum(out=grs[:, m, 1:2], in_=t_,
                                             axis=AX.X)

                def dist_tail():
                    pr = psd.tile([BC, 2], F32, tag="d")
                    for dm in range(MR):
                        nc.tensor.matmul(pr, sel8, grs[:, dm, :],
                                         start=(dm == 0), stop=(dm == MR - 1))
                    nc.scalar.activation(gsb, pr[:, 0:1], AF.Sigmoid,
                                         bias=b3bc[:, 1:2], scale=1.0 / S)
                    nc.scalar.activation(dsb, pr[:, 1:2], AF.Sigmoid,
                                         bias=b3bc[:, 2:3])

                if not KFILL:
                    for dm in range(MR):
                        dist_half(dm, 0)
                        dist_half(dm, 1)
                        dist_finish(dm)
                    dist_tail()

                OW = 32 if KWH == "mix" else BC

                def emit_folds(s):
                    # xg+bias fold matmuls for step s: independent of h, so
                    # they are emitted one step ahead (into the PE stream
                    # ahead of step s-1's transposes) to fill the PE stall
                    # while act(o)/h of the previous step complete.
                    m = s // 16
                    p0 = (s * 8) % 128
                    blk = (p0 // 64) * 64
                    q = (p0 % 64) // 8
                    pss = {}
                    for nb in NBORD:
                        ps = psl.tile([OW, 512], F32, tag="l", name=f"l{s%4}")
                        nhalf = 2 if (nb == 2 and KOSPLIT) else 1
                        w = 512 // nhalf
                        for hh in range(nhalf):
                            nc.tensor.matmul(
                                ps[:, hh * w:(hh + 1) * w],
                                eyevar[blk:blk + 64, q, 0:OW],
                                xgsb[blk:blk + 64, m,
                                     nb * 512 + hh * w:nb * 512 + (hh + 1) * w],
                                start=True, stop=(s == 0))
                        pss[nb] = ps
                    return pss

                pss_next = emit_folds(0)
                for s in range(S):
                    pss = pss_next
                    ga = {}
                    for nb in NBORD:
                        ps = pss[nb]
                        nhalf = 2 if (nb == 2 and KOSPLIT) else 1
                        w = 512 // nhalf
                        for hh in range(nhalf):
                            if s > 0 and KWH == "mix":
                                # k-tiles 0,1 in one fp8 DoubleRow pass
                                nc.tensor.matmul(
                                    ps[:, hh * w:(hh + 1) * w],
                                    hT8,
                                    wh8sb[:, :, nb * 512 + hh * w:
                                          nb * 512 + (hh + 1) * w],
                                    start=False, stop=False,
                                    perf_mode=mybir.MatmulPerfMode.DoubleRow)
                                for k in (2, 3):
                                    nc.tensor.matmul(
                                        ps[0:BC, hh * w:(hh + 1) * w],
                                        hT[:, k, 0:BC],
                                        whsb[:, k, nb * 512 + hh * w:
                                             nb * 512 + (hh + 1) * w],
                                        start=False, stop=(k == KH - 1),
                                        skip_group_check=True)
                            elif s > 0:
                                for k in range(KH):
                                    nc.tensor.matmul(
                                        ps[0:BC, hh * w:(hh + 1) * w],
                                        hT[:, k, 0:BC],
                                        whsb[:, k, nb * 512 + hh * w:
                                             nb * 512 + (hh + 1) * w],
                                        start=False, stop=(k == KH - 1))
                            gpool = gap if nhalf == 1 else gaop
                            g = gpool.tile([BC, w], BF)
                            nc.scalar.activation(
                                g, ps[0:BC, hh * w:(hh + 1) * w],
                                AF.Tanh if nb == 3 else AF.Sigmoid,
                                scale=1.0 / WH_SCALE)
                            ga.setdefault(nb, []).append(g)
                    if s == 0:
                        nc.vector.tensor_mul(cst, ga[0][0], ga[3][0])
                    else:
                        t1 = ltp.tile([BC, H], BF)
                        nc.vector.tensor_mul(t1, ga[0][0], ga[3][0])  # i*g
                        nc.vector.tensor_mul(cst, cst, ga[1][0])      # f*c
                        nc.vector.tensor_add(cst, cst, t1)
                    if s + 1 < S:
                        pss_next = emit_folds(s + 1)
                    # PE filler between the step's matmuls and transposes:
                    # real work where available, else p-state keepalive.
                    fins = []
                    if KFILL:
                        if s < 12:
                            fins.append(xg_group(psx, 1 + s // 4, s % 4))
                        elif s < 20:
                            dm, dh = (s - 12) // 2, (s - 12) % 2
                            dist_half(dm, dh)
                            if dh == 1:
                                fins.append(lambda dm=dm: dist_finish(dm))
                        else:
                            for _ in range(KDUM):
                                dps = psx.tile([128, 512], F32, tag="x")
                                nc.tensor.matmul(dps, fuseT[:, 0, 0:128],
                                                 wxp[:, 0, 0:512],
                                                 start=True, stop=True)
                    # h = o*c in k-chunks so transpose/cast pipeline per k;
                    # casts alternate DVE/Act to halve the serial tail.
                    h = hp.tile([BC, H], BF)
                    for k in range(KH):
                        if len(ga[2]) == 2:
                            osrc = ga[2][k // 2][:, (k % 2) * 128:
                                                 (k % 2 + 1) * 128]
                        else:
                            osrc = ga[2][0][:, k * 128:(k + 1) * 128]
                        nc.vector.tensor_mul(
                            h[:, k * 128:(k + 1) * 128], osrc,
                            cst[:, k * 128:(k + 1) * 128])
                        pt = pstr.tile([128, BC], BF, tag="tr")
                        nc.tensor.transpose(pt, h[:, k * 128:(k + 1) * 128],
                                            eye8)
                        # all casts on DVE: a cast on the in-order act engine
                        # would block the next step's gate activations
                        if KWH == "mix" and k < 2:
                            nc.vector.tensor_copy(hT8[:, k, 0:BC], pt)
                        else:
                            nc.vector.tensor_copy(hT[:, k, 0:BC], pt)
                    for fcb in fins:
                        fcb()
                    if KFILL and s == 21:
                        dist_tail()
                # decoder
                pd = psd.tile([BC, 2], F32, tag="d")
                for k in range(KH):
                    if KWH == "mix" and k < 2:
                        nc.tensor.matmul(pd[:, 0:1], hT8[:, k, 0:BC],
                                         wdec8p[:, k:k + 1],
                                         start=(k == 0), stop=False)
                    else:
                        nc.tensor.matmul(pd[:, 0:1], hT[:, k, 0:BC],
                                         wdecp[:, k:k + 1],
                                         start=(k == 0), stop=(k == KH - 1))
                nc.scalar.activation(osb, pd[:, 0:1], AF.Sigmoid,
                                     bias=b3bc[:, 0:1], scale=1.0 / WH_SCALE)

        # ---- combine ----
        if PHASES >= 6:
            nc.vector.tensor_sub(fin, osb, dsb)
            nc.vector.scalar_tensor_tensor(fin, fin, gsb[:, 0:1], dsb,
                                           ALU.mult, ALU.add)
            nc.sync.dma_start(out_d, fin)


_NC_CACHE = {}


def _get_nc():
    if "nc" not in _NC_CACHE:
        _NC_CACHE["nc"] = build_nc()
    return _NC_CACHE["nc"]


def _make_in_maps(v_feat, category, W_enc, b_enc, Wx, bx, Wh, bh, cat_emb,
                  W_dec, b_dec, prototype, W_dd, b_dd, W_gate, b_gate):
    f32 = np.float32
    v_feat = np.asarray(v_feat, f32)
    category = np.asarray(category).astype(np.int64)

    wencp = np.ascontiguousarray(
        np.asarray(W_enc, f32).reshape(KF, 128, H).transpose(1, 0, 2)
    ).astype(BF_NP)
    benc = np.ascontiguousarray(
        np.asarray(b_enc, f32).reshape(KH, 128).T).copy()
    catemb = np.asarray(cat_emb, f32).astype(BF_NP)
    wxp = np.ascontiguousarray(
        np.asarray(Wx, f32).reshape(KH, 128, G).transpose(1, 0, 2)
    ).astype(BF_NP)
    bxh128 = np.ascontiguousarray(
        np.tile(WH_SCALE * (np.asarray(bx, f32)
                            + np.asarray(bh, f32)).reshape(1, G),
                (128, 1)))
    whs = (WH_SCALE * np.asarray(Wh, f32)).reshape(KH, 128, G)
    whp8 = np.ascontiguousarray(whs.transpose(1, 0, 2)).astype(BF_NP)
    wh8 = np.ascontiguousarray(whs[0:2].transpose(1, 0, 2)).astype(FP8_NP)
    # eyevar[p, q, j] = 1 iff p%64 == q*8+j (64-aligned step-row selector);
    # cols 8..31 are zero padding so the 32-wide PSUM region is fully started.
    pp_ = np.arange(128)
    eyevar = np.zeros((128, 8, 32), f32)
    for qq in range(8):
        for j in range(BC):
            eyevar[pp_ % 64 == qq * 8 + j, qq, j] = 1.0
    eyevar = eyevar.astype(BF_NP)
    eye8 = np.eye(BC, dtype=f32).astype(BF_NP)
    proto = np.asarray(prototype, f32)
    pw = np.concatenate([-2.0 * proto,
                         np.asarray(W_gate, f32).reshape(1, F),
                         np.zeros((1, F), f32)], axis=0)  # [22, F]
    pwT = np.ascontiguousarray(
        pw.T.reshape(KF, 128, 22).transpose(1, 0, 2)).astype(BF_NP)
    pprow = np.concatenate([(proto * proto).sum(axis=1),
                            np.zeros(2, f32)]).reshape(1, 22).astype(f32)
    ones1 = np.ones((1, 128), f32)
    wdd = np.asarray(W_dd, f32).reshape(S, P2)
    rep = np.repeat(wdd[:, None, :], BC, axis=1).reshape(R, P2)
    wddr = np.ascontiguousarray(rep.reshape(MR, 128, P2).transpose(1, 0, 2))
    sel8 = np.zeros((128, BC), f32)
    sel8[np.arange(128), np.arange(128) % BC] = 1.0
    wdecs = (WH_SCALE * np.asarray(W_dec, f32)).reshape(KH, 128).T
    wdecp8 = np.ascontiguousarray(wdecs).astype(BF_NP)
    wdec8 = np.ascontiguousarray(wdecs[:, 0:2]).astype(FP8_NP)
    b3 = np.array([np.asarray(b_dec, f32).reshape(-1)[0],
                   np.asarray(b_gate, f32).reshape(-1)[0],
                   np.asarray(b_dd, f32).reshape(-1)[0]], f32)
    b3bc = np.ascontiguousarray(np.tile(b3.reshape(1, 3), (BC, 1)))

    common = {
        "wencp": wencp, "benc": benc, "catemb": catemb, "wxp": wxp,
        "bxh128": bxh128, "whp8": whp8, "wh8": wh8, "wdec8": wdec8,
        "eyevar": eyevar, "eye8": eye8,
        "pwT": pwT, "pprow": pprow, "ones1": ones1, "wddr": wddr,
        "sel8": sel8, "wdecp8": wdecp8, "b3bc": b3bc,
    }
    in_maps = []
    for j in range(NCORES):
        vs = np.ascontiguousarray(
            v_feat[j * BC:(j + 1) * BC].transpose(1, 0, 2).reshape(R, F))
        vTn = np.ascontiguousarray(
            vs.reshape(R, KF, 128).transpose(2, 1, 0)).astype(BF_NP)
        vrow = np.ascontiguousarray(
            vs.reshape(MR, 128, F).transpose(1, 0, 2)).astype(BF_NP)
        cats = category[j * BC:(j + 1) * BC]
        onehot = (cats[None, :] == np.arange(3)[:, None]).astype(f32)
        onehot = np.ascontiguousarray(
            np.tile(onehot, (1, S))).astype(BF_NP)  # [3, R], r = s*8+b
        in_maps.append({"vT": vTn, "vrow": vrow, "onehot": onehot, **common})
    return in_maps


def run(trace=False, **inputs):
    nc = _get_nc()
    in_maps = _make_in_maps(**inputs)
    res = run_bass_kernel_spmd(nc, in_maps, list(range(NCORES)), trace=trace)
    out = np.concatenate([res.results[j]["out"] for j in range(NCORES)],
                         axis=0).astype(np.float32)
    return out, res


def kernel(**inputs):
    out, _ = run(trace=False, **inputs)
    return out


# revision 92
# speedup vs baseline: 1.2281x; 1.0065x over previous
"""Trainium2 Bass kernel for nn_ASD_RNN (encoder + fused-gate LSTM + prototype-distance head).

Contract: kernel(**inputs) takes FULL unsharded inputs (as in reference.setup_inputs())
and returns the FULL [64, 1] float32 output. Internally shards batch across 8 cores
(8 batches per core), runs one Bass kernel per core via run_bass_kernel_spmd, gathers.

Per-core layout (BC = 8 batches, R = BC*S = 512 rows, s-major: r = s*8 + b):
  - All GEMM operands are bf16; PSUM accumulation is fp32.
  - v is shipped pre-transposed from the host: vT [f%128, f//128, r] feeds the
    encoder and prototype-distance GEMMs; a row-major copy vrow feeds ||v||^2.
  - Encoder: fuseT[h%128, h//128, r] = relu(Wenc^T v + benc) + cat_emb one-hot fold.
  - xg = fuse @ Wx + (bx+bh), computed row-major into SBUF-resident xgsb
    [r%128, r//128, g] (bias added via a host-replicated [128, G] tensor).
  - LSTM keeps h transposed (hT [h%128, k, b]) as the matmul stationary; Wh is the
    moving operand (one full Wh pass per step is the PE floor). The per-step xg+bias
    contribution is folded into the gates PSUM with a 32-row selector matmul
    (eyevar) so no per-step DMA or staging copies are needed.
  - Distance head: ||v-p||^2 via matmul folds (-2p | W_gate rides as column 20),
    log-ratio via two Ln activations with per-partition bias, reduced via a
    selector matmul.
"""

import numpy as np
import ml_dtypes

import concourse.bass as bass
import concourse.mybir as mybir
import concourse.tile as tile
from concourse import bacc
from concourse.bass_utils import run_bass_kernel_spmd

AF = mybir.ActivationFunctionType
ALU = mybir.AluOpType
DT = mybir.dt
AX = mybir.AxisListType

B, S, F, H, P2 = 64, 64, 2048, 512, 20
G = 4 * H
NCORES = 8
BC = B // NCORES          # 8 batches per core
R = BC * S                # 512 rows per core
KF = F // 128             # 16 k-tiles over F
KH = H // 128             # 4 k-tiles over H
MR = R // 128             # 4 row tiles
F32 = DT.float32
BF = DT.bfloat16
FP8 = DT.float8e4
MMF = DT.float32r
BF_NP = ml_dtypes.bfloat16
FP8_NP = ml_dtypes.float8_e4m3
WH_SCALE = 32.0  # Wh/W_dec stored fp8 as x32; pre-acts carry x32, acts undo

import os as _os
# "bf16" (default): fully unthrottled, fastest measured. "mix" (k-tiles 0-1
# via fp8-DoubleRow) cuts the Wh stream 25% but still trips the PE power
# limiter and measures slower.
KWH = _os.environ.get("KWH", "bf16")
KDUM = int(_os.environ.get("KDUM", "0"))
KOSPLIT = int(_os.environ.get("KOSPLIT", "0"))
KFILL = int(_os.environ.get("KFILL", "0"))


def build_nc():
    nc = bacc.Bacc("TRN2", target_bir_lowering=False, debug=False,
                   num_devices=NCORES)

    def din(name, shape, dt=BF):
        return nc.dram_tensor(name, shape, dt, kind="ExternalInput").ap()

    vT_d = din("vT", [128, KF, R])
    vrow_d = din("vrow", [128, MR, F])
    wenc_d = din("wencp", [128, KF, H])
    benc_d = din("benc", [128, KH], F32)
    catemb_d = din("catemb", [3, H])
    onehot_d = din("onehot", [3, R])
    wx_d = din("wxp", [128, KH, G])
    bxh_d = din("bxh128", [128, G], F32)
    wh_d = din("whp8", [128, KH, G], BF)
    wh8_d = din("wh8", [128, 2, G], FP8)
    wdec8_d = din("wdec8", [128, 2], FP8)
    eyevar_d = din("eyevar", [128, 8, 32])
    eye8_d = din("eye8", [BC, BC])
    pwT_d = din("pwT", [128, KF, 22])
    pprow_d = din("pprow", [1, 22], MMF)
    ones1_d = din("ones1", [1, 128], MMF)
    wddr_d = din("wddr", [128, MR, P2], F32)
    sel8_d = din("sel8", [128, BC], MMF)
    wdec_d = din("wdecp8", [128, KH], BF)
    b3bc_d = din("b3bc", [BC, 3], F32)
    out_d = nc.dram_tensor("out", [BC, 1], F32, kind="ExternalOutput").ap()

    with tile.TileContext(nc) as tc:
        _body(tc, nc, vT_d, vrow_d, wenc_d, benc_d, catemb_d, onehot_d, wx_d,
              bxh_d, wh_d, wh8_d, wdec8_d, eyevar_d, eye8_d, pwT_d, pprow_d,
              ones1_d, wddr_d, sel8_d, wdec_d, b3bc_d, out_d)
    nc.compile()
    return nc


def _body(tc, nc, vT_d, vrow_d, wenc_d, benc_d, catemb_d, onehot_d, wx_d,
          bxh_d, wh_d, wh8_d, wdec8_d, eyevar_d, eye8_d, pwT_d, pprow_d,
          ones1_d, wddr_d, sel8_d, wdec_d, b3bc_d, out_d):
    import os
    PHASES = int(os.environ.get("KPHASES", "9"))
    with tc.tile_pool(name="persist", bufs=1) as P:
        vT = P.tile([128, KF, R], BF)
        wencp = P.tile([128, KF, H], BF)
        wxp = P.tile([128, KH, G], BF)
        whsb = P.tile([128, KH, G], BF)
        if KWH == "mix":
            wh8sb = P.tile([128, 2, G], FP8)
            wdec8p = P.tile([128, 2], FP8)
        fuseT = P.tile([128, KH, R], BF)
        xgsb = P.tile([128, MR, G], BF)
        bxh128 = P.tile([128, G], F32)
        catemb = P.tile([3, H], BF)
        onehot = P.tile([3, R], BF)
        benc = P.tile([128, KH], F32)
        pwT = P.tile([128, KF, 22], BF)
        pprow = P.tile([1, 22], MMF)
        ones1f = P.tile([1, 128], MMF)
        wddr = P.tile([128, MR, P2], F32)
        sel8 = P.tile([128, BC], MMF)
        eyevar = P.tile([128, 8, 32], BF)
        eye8 = P.tile([BC, BC], BF)
        wdecp = P.tile([128, KH], BF)
        b3bc = P.tile([BC, 3], F32)
        hT = P.tile([128, KH, BC], BF)
        if KWH == "mix":
            # 32-wide padding: DoubleRow needs >=32 stationary cols
            hT8 = P.tile([128, 2, 32], FP8)
        cst = P.tile([BC, H], BF)
        vv = P.tile([128, MR], F32)
        vve = P.tile([128, MR], F32)
        grs = P.tile([128, MR, 2], MMF)
        osb = P.tile([BC, 1], F32)
        gsb = P.tile([BC, 1], F32)
        dsb = P.tile([BC, 1], F32)
        fin = P.tile([BC, 1], F32)

        # ---- input DMAs (queue order = priority: encoder set first; vT and
        # wencp chunked by ko so the encoder k-loop can start early) ----
        for c in range(4):
            nc.sync.dma_start(vT[:, 4 * c:4 * (c + 1), :],
                              vT_d[:, 4 * c:4 * (c + 1), :])
        for c in range(2):
            nc.sync.dma_start(wencp[:, 8 * c:8 * (c + 1), :],
                              wenc_d[:, 8 * c:8 * (c + 1), :])
        nc.sync.dma_start(benc, benc_d)
        nc.sync.dma_start(catemb, catemb_d)
        nc.sync.dma_start(onehot, onehot_d)
        nc.sync.dma_start(wxp, wx_d)
        nc.sync.dma_start(bxh128, bxh_d)
        nc.sync.dma_start(whsb, wh_d)
        if KWH == "mix":
            nc.sync.dma_start(wh8sb, wh8_d)
            nc.sync.dma_start(wdec8p, wdec8_d)
        nc.sync.dma_start(eyevar, eyevar_d)
        nc.sync.dma_start(eye8, eye8_d)
        nc.sync.dma_start(pwT, pwT_d)
        nc.sync.dma_start(pprow, pprow_d)
        nc.sync.dma_start(ones1f, ones1_d)
        nc.sync.dma_start(wddr, wddr_d)
        nc.sync.dma_start(sel8, sel8_d)
        nc.sync.dma_start(wdecp, wdec_d)
        nc.sync.dma_start(b3bc, b3bc_d)

        # ---- encoder: fuseT = relu(Wenc^T v + benc) + catemb-fold ----
        if PHASES >= 2:
            with tc.tile_pool(name="psf", bufs=2, space="PSUM") as psf, \
                 tc.tile_pool(name="psc", bufs=2, space="PSUM") as psc, \
                 tc.tile_pool(name="encs", bufs=2) as encs:
                for m in range(KH):
                    ps = psf.tile([128, R], F32)
                    for ko in range(KF):
                        nc.tensor.matmul(
                            ps, wencp[:, ko, m * 128:(m + 1) * 128],
                            vT[:, ko, :], start=(ko == 0), stop=(ko == KF - 1))
                    pc = psc.tile([128, R], F32)
                    nc.tensor.matmul(pc, catemb[:, m * 128:(m + 1) * 128],
                                     onehot, start=True, stop=True)
                    sc = encs.tile([128, R], F32)
                    nc.scalar.activation(sc, ps, AF.Relu,
                                         bias=benc[:, m:m + 1])
                    nc.vector.tensor_add(fuseT[:, m, :], sc, pc)

        # ---- vv[r] = sum_f v[r,f]^2 (Act engine; emitted after the encoder
        # relus so a late vrow DMA can't stall them on the in-order engine) --
        if PHASES >= 1:
            with tc.tile_pool(name="vvp", bufs=1) as VP, \
                 tc.tile_pool(name="vsq", bufs=2) as SQ:
                vrow = VP.tile([128, MR, F], BF)
                nc.sync.dma_start(vrow, vrow_d)
                for m in range(MR):
                    sq = SQ.tile([128, F], BF)
                    nc.scalar.activation(sq, vrow[:, m, :], AF.Square,
                                         accum_out=vv[:, m:m + 1])
                nc.vector.tensor_scalar_add(vve, vv, 1e-8)

        # ---- xg row-block m: 4 psum groups + x32 bias fold (shared) ----
        def xg_group(psx, m, nb):
            ps = psx.tile([128, 512], F32, tag="x")
            for k in range(KH):
                nc.tensor.matmul(
                    ps, fuseT[:, k, m * 128:(m + 1) * 128],
                    wxp[:, k, nb * 512:(nb + 1) * 512],
                    start=(k == 0), stop=(k == KH - 1))
            # xg carries x32 so it folds into the x32 fp8 Wh partials;
            # bxh128 is host-prescaled by 32.
            def fin():
                nc.vector.scalar_tensor_tensor(
                    xgsb[:, m, nb * 512:(nb + 1) * 512], ps, WH_SCALE,
                    bxh128[:, nb * 512:(nb + 1) * 512], ALU.mult, ALU.add)
            return fin

        # ---- xg ahead of the LSTM (m>0 interleaves into steps if KFILL) ----
        if PHASES >= 3:
            with tc.tile_pool(name="psx0", bufs=2, space="PSUM") as psx0:
                for mm_ in range(1 if KFILL else MR):
                    for nb in range(4):
                        xg_group(psx0, mm_, nb)()

        # ---- LSTM + interleaved distance head + xg m=1..3 + decoder ----
        if PHASES >= 5:
            NBORD = (0, 1, 3, 2)  # i, f, g, o last: c-chain unblocks earlier
            nc.vector.memset(hT.bitcast(DT.uint8), 0)
            if KWH == "mix":
                nc.vector.memset(hT8.bitcast(DT.uint8), 0)
            with tc.tile_pool(name="psl", bufs=3 if KFILL else 4,
                              space="PSUM") as psl, \
                 tc.tile_pool(name="pstr", bufs=2, space="PSUM") as pstr, \
                 tc.tile_pool(name="psx", bufs=1, space="PSUM") as psx, \
                 tc.tile_pool(name="psq", bufs=1, space="PSUM") as psq, \
                 tc.tile_pool(name="psd", bufs=1, space="PSUM") as psd, \
                 tc.tile_pool(name="gap", bufs=6) as gap, \
                 tc.tile_pool(name="gaop", bufs=4) as gaop, \
                 tc.tile_pool(name="hp", bufs=2) as hp, \
                 tc.tile_pool(name="dfp", bufs=6) as dfp, \
                 tc.tile_pool(name="ltp", bufs=2) as ltp:
                dist_pq = [None] * MR

                def dist_half(m, half):
                    # 8 of the 16 -2*v.p matmuls for row-block m (PE filler)
                    if half == 0:
                        dist_pq[m] = psq.tile([128, 22], F32, tag="q",
                                              name=f"pq{m}")
                    pq = dist_pq[m]
                    for ko in range(8 * half, 8 * half + 8):
                        nc.tensor.matmul(pq, vT[:, ko, m * 128:(m + 1) * 128],
                                         pwT[:, ko, :],
                                         start=(ko == 0), stop=False)
                    if half == 1:
                        nc.tensor.matmul(pq, ones1f, pprow,
                                         start=False, stop=True)

                def dist_finish(m):
                    # dist_feat = log((d+1)/(d+eps)) ~= x - x^2/2, x = 1/d
                    # (d ~ 2700 so the truncation error is ~1e-11); keeps the
                    # act engine on the sigmoid table (no Ln table swap).
                    pq = dist_pq[m]
                    dd = dfp.tile([128, P2], F32)
                    nc.scalar.activation(dd, pq[:, 0:P2], AF.Identity,
                                         bias=vve[:, m:m + 1])
                    nc.scalar.copy(grs[:, m, 0:1], pq[:, P2:P2 + 1])
                    x_ = dfp.tile([128, P2], F32)
                    nc.vector.reciprocal(x_, dd)
                    t_ = dfp.tile([128, P2], F32)
                    nc.vector.tensor_scalar_mul(t_, x_, -0.5)
                    nc.vector.tensor_scalar_add(t_, t_, 1.0)
                    nc.vector.tensor_mul(t_, t_, x_)
                    nc.vector.tensor_mul(t_, t_, wddr[:, m, :])
                    with nc.allow_low_precision(reason="20-wide reduce, f32r"):
                        nc.vector.reduce_sum(out=grs[:, m, 1:2], in_=t_,
                                             axis=AX.X)

                def dist_tail():
                    pr = psd.tile([BC, 2], F32, tag="d")
                    for dm in range(MR):
                        nc.tensor.matmul(pr, sel8, grs[:, dm, :],
                                         start=(dm == 0), stop=(dm == MR - 1))
                    nc.scalar.activation(gsb, pr[:, 0:1], AF.Sigmoid,
                                         bias=b3bc[:, 1:2], scale=1.0 / S)
                    nc.scalar.activation(dsb, pr[:, 1:2], AF.Sigmoid,
                                         bias=b3bc[:, 2:3])

                if not KFILL:
                    for dm in range(MR):
                        dist_half(dm, 0)
                        dist_half(dm, 1)
                        dist_finish(dm)
                    dist_tail()

                OW = 32 if KWH == "mix" else BC

                def emit_folds(s):
                    # xg+bias fold matmuls for step s: independent of h, so
                    # they are emitted one step ahead (into the PE stream
                    # ahead of step s-1's transposes) to fill the PE stall
                    # while act(o)/h of the previous step complete.
                    m = s // 16
                    p0 = (s * 8) % 128
                    blk = (p0 // 64) * 64
                    q = (p0 % 64) // 8
                    pss = {}
                    for nb in NBORD:
                        ps = psl.tile([OW, 512], F32, tag="l", name=f"l{s%4}")
                        nhalf = 2 if (nb == 2 and KOSPLIT) else 1
                        w = 512 // nhalf
                        for hh in range(nhalf):
                            nc.tensor.matmul(
                                ps[:, hh * w:(hh + 1) * w],
                                eyevar[blk:blk + 64, q, 0:OW],
                                xgsb[blk:blk + 64, m,
                                     nb * 512 + hh * w:nb * 512 + (hh + 1) * w],
                                start=True, stop=(s == 0))
                        pss[nb] = ps
                    return pss

                pss_next = emit_folds(0)
                for s in range(S):
                    pss = pss_next
                    ga = {}
                    for nb in NBORD:
                        ps = pss[nb]
                        nhalf = 2 if (nb == 2 and KOSPLIT) else 1
                        w = 512 // nhalf
                        for hh in range(nhalf):
                            if s > 0 and KWH == "mix":
                                # k-tiles 0,1 in one fp8 DoubleRow pass
                                nc.tensor.matmul(
                                    ps[:, hh * w:(hh + 1) * w],
                                    hT8,
                                    wh8sb[:, :, nb * 512 + hh * w:
                                          nb * 512 + (hh + 1) * w],
                                    start=False, stop=False,
                                    perf_mode=mybir.MatmulPerfMode.DoubleRow)
                                for k in (2, 3):
                                    nc.tensor.matmul(
                                        ps[0:BC, hh * w:(hh + 1) * w],
                                        hT[:, k, 0:BC],
                                        whsb[:, k, nb * 512 + hh * w:
                                             nb * 512 + (hh + 1) * w],
                                        start=False, stop=(k == KH - 1),
                                        skip_group_check=True)
                            elif s > 0:
                                for k in range(KH):
                                    nc.tensor.matmul(
                                        ps[0:BC, hh * w:(hh + 1) * w],
                                        hT[:, k, 0:BC],
                                        whsb[:, k, nb * 512 + hh * w:
                                             nb * 512 + (hh + 1) * w],
                                        start=False, stop=(k == KH - 1))
                            gpool = gap if nhalf == 1 else gaop
                            g = gpool.tile([BC, w], BF)
                            nc.scalar.activation(
                                g, ps[0:BC, hh * w:(hh + 1) * w],
                                AF.Tanh if nb == 3 else AF.Sigmoid,
                                scale=1.0 / WH_SCALE)
                            ga.setdefault(nb, []).append(g)
                    if s == 0:
                        nc.vector.tensor_mul(cst, ga[0][0], ga[3][0])
                    else:
                        t1 = ltp.tile([BC, H], BF)
                        nc.vector.tensor_mul(t1, ga[0][0], ga[3][0])  # i*g
                        nc.vector.tensor_mul(cst, cst, ga[1][0])      # f*c
                        nc.vector.tensor_add(cst, cst, t1)
                    if s + 1 < S:
                        pss_next = emit_folds(s + 1)
                    # PE filler between the step's matmuls and transposes:
                    # real work where available, else p-state keepalive.
                    fins = []
                    if KFILL:
                        if s < 12:
                            fins.append(xg_group(psx, 1 + s // 4, s % 4))
                        elif s < 20:
                            dm, dh = (s - 12) // 2, (s - 12) % 2
                            dist_half(dm, dh)
                            if dh == 1:
                                fins.append(lambda dm=dm: dist_finish(dm))
                        else:
                            for _ in range(KDUM):
                                dps = psx.tile([128, 512], F32, tag="x")
                                nc.tensor.matmul(dps, fuseT[:, 0, 0:128],
                                                 wxp[:, 0, 0:512],
                                                 start=True, stop=True)
                    # h = o*c in k-chunks so transpose/cast pipeline per k;
                    # casts alternate DVE/Act to halve the serial tail.
                    h = hp.tile([BC, H], BF)
                    for k in range(KH):
                        if len(ga[2]) == 2:
                            osrc = ga[2][k // 2][:, (k % 2) * 128:
                                                 (k % 2 + 1) * 128]
                        else:
                            osrc = ga[2][0][:, k * 128:(k + 1) * 128]
                        nc.vector.tensor_mul(
                            h[:, k * 128:(k + 1) * 128], osrc,
                            cst[:, k * 128:(k + 1) * 128])
                        pt = pstr.tile([128, BC], BF, tag="tr")
                        nc.tensor.transpose(pt, h[:, k * 128:(k + 1) * 128],
                                            eye8)
                        # all casts on DVE: a cast on the in-order act engine
                        # would block the next step's gate activations
                        if KWH == "mix" and k < 2:
                            nc.vector.tensor_copy(hT8[:, k, 0:BC], pt)
                        else:
                            nc.vector.tensor_copy(hT[:, k, 0:BC], pt)
                    for fcb in fins:
                        fcb()
                    if KFILL and s == 21:
                        dist_tail()
                # decoder
                pd = psd.tile([BC, 2], F32, tag="d")
                for k in range(KH):
                    if KWH == "mix" and k < 2:
                        nc.tensor.matmul(pd[:, 0:1], hT8[:, k, 0:BC],
                                         wdec8p[:, k:k + 1],
                                         start=(k == 0), stop=False)
                    else:
                        nc.tensor.matmul(pd[:, 0:1], hT[:, k, 0:BC],
                                         wdecp[:, k:k + 1],
                                         start=(k == 0), stop=(k == KH - 1))
                nc.scalar.activation(osb, pd[:, 0:1], AF.Sigmoid,
                                     bias=b3bc[:, 0:1], scale=1.0 / WH_SCALE)

        # ---- combine ----
        if PHASES >= 6:
            nc.vector.tensor_sub(fin, osb, dsb)
            nc.vector.scalar_tensor_tensor(fin, fin, gsb[:, 0:1], dsb,
                                           ALU.mult, ALU.add)
            nc.sync.dma_start(out_d, fin)


_NC_CACHE = {}


def _get_nc():
    if "nc" not in _NC_CACHE:
        _NC_CACHE["nc"] = build_nc()
    return _NC_CACHE["nc"]


def _make_in_maps(v_feat, category, W_enc, b_enc, Wx, bx, Wh, bh, cat_emb,
                  W_dec, b_dec, prototype, W_dd, b_dd, W_gate, b_gate):
    f32 = np.float32
    v_feat = np.asarray(v_feat, f32)
    category = np.asarray(category).astype(np.int64)

    wencp = np.ascontiguousarray(
        np.asarray(W_enc, f32).reshape(KF, 128, H).transpose(1, 0, 2)
    ).astype(BF_NP)
    benc = np.ascontiguousarray(
        np.asarray(b_enc, f32).reshape(KH, 128).T).copy()
    catemb = np.asarray(cat_emb, f32).astype(BF_NP)
    wxp = np.ascontiguousarray(
        np.asarray(Wx, f32).reshape(KH, 128, G).transpose(1, 0, 2)
    ).astype(BF_NP)
    bxh128 = np.ascontiguousarray(
        np.tile(WH_SCALE * (np.asarray(bx, f32)
                            + np.asarray(bh, f32)).reshape(1, G),
                (128, 1)))
    whs = (WH_SCALE * np.asarray(Wh, f32)).reshape(KH, 128, G)
    whp8 = np.ascontiguousarray(whs.transpose(1, 0, 2)).astype(BF_NP)
    wh8 = np.ascontiguousarray(whs[0:2].transpose(1, 0, 2)).astype(FP8_NP)
    # eyevar[p, q, j] = 1 iff p%64 == q*8+j (64-aligned step-row selector);
    # cols 8..31 are zero padding so the 32-wide PSUM region is fully started.
    pp_ = np.arange(128)
    eyevar = np.zeros((128, 8, 32), f32)
    for qq in range(8):
        for j in range(BC):
            eyevar[pp_ % 64 == qq * 8 + j, qq, j] = 1.0
    eyevar = eyevar.astype(BF_NP)
    eye8 = np.eye(BC, dtype=f32).astype(BF_NP)
    proto = np.asarray(prototype, f32)
    pw = np.concatenate([-2.0 * proto,
                         np.asarray(W_gate, f32).reshape(1, F),
                         np.zeros((1, F), f32)], axis=0)  # [22, F]
    pwT = np.ascontiguousarray(
        pw.T.reshape(KF, 128, 22).transpose(1, 0, 2)).astype(BF_NP)
    pprow = np.concatenate([(proto * proto).sum(axis=1),
                            np.zeros(2, f32)]).reshape(1, 22).astype(f32)
    ones1 = np.ones((1, 128), f32)
    wdd = np.asarray(W_dd, f32).reshape(S, P2)
    rep = np.repeat(wdd[:, None, :], BC, axis=1).reshape(R, P2)
    wddr = np.ascontiguousarray(rep.reshape(MR, 128, P2).transpose(1, 0, 2))
    sel8 = np.zeros((128, BC), f32)
    sel8[np.arange(128), np.arange(128) % BC] = 1.0
    wdecs = (WH_SCALE * np.asarray(W_dec, f32)).reshape(KH, 128).T
    wdecp8 = np.ascontiguousarray(wdecs).astype(BF_NP)
    wdec8 = np.ascontiguousarray(wdecs[:, 0:2]).astype(FP8_NP)
    b3 = np.array([np.asarray(b_dec, f32).reshape(-1)[0],
                   np.asarray(b_gate, f32).reshape(-1)[0],
                   np.asarray(b_dd, f32).reshape(-1)[0]], f32)
    b3bc = np.ascontiguousarray(np.tile(b3.reshape(1, 3), (BC, 1)))

    common = {
        "wencp": wencp, "benc": benc, "catemb": catemb, "wxp": wxp,
        "bxh128": bxh128, "whp8": whp8, "wh8": wh8, "wdec8": wdec8,
        "eyevar": eyevar, "eye8": eye8,
        "pwT": pwT, "pprow": pprow, "ones1": ones1, "wddr": wddr,
        "sel8": sel8, "wdecp8": wdecp8, "b3bc": b3bc,
    }
    in_maps = []
    for j in range(NCORES):
        vs = np.ascontiguousarray(
            v_feat[j * BC:(j + 1) * BC].transpose(1, 0, 2).reshape(R, F))
        vTn = np.ascontiguousarray(
            vs.reshape(R, KF, 128).transpose(2, 1, 0)).astype(BF_NP)
        vrow = np.ascontiguousarray(
            vs.reshape(MR, 128, F).transpose(1, 0, 2)).astype(BF_NP)
        cats = category[j * BC:(j + 1) * BC]
        onehot = (cats[None, :] == np.arange(3)[:, None]).astype(f32)
        onehot = np.ascontiguousarray(
            np.tile(onehot, (1, S))).astype(BF_NP)  # [3, R], r = s*8+b
        in_maps.append({"vT": vTn, "vrow": vrow, "onehot": onehot, **common})
    return in_maps


def run(trace=False, **inputs):
    nc = _get_nc()
    in_maps = _make_in_maps(**inputs)
    res = run_bass_kernel_spmd(nc, in_maps, list(range(NCORES)), trace=trace)
    out = np.concatenate([res.results[j]["out"] for j in range(NCORES)],
                         axis=0).astype(np.float32)
    return out, res


def kernel(**inputs):
    out, _ = run(trace=False, **inputs)
    return out
